# revision 1
# baseline (speedup 1.0000x reference)
# Distributed sparse-attention kernel for Trainium2 (8 NeuronCores).
#
# Sharding: core c = (batch b = c//2, head-group g = c%2 of 8 heads).
# Each core computes, for its (b, g):
#   q  = meancenter(x) @ Wc          (LN rstd cancels under l2norm; gamma and
#                                     mean-centering folded into Wc on host)
#   kv = [prefix; x] @ Wkv.T         (MQA single head, replicated per batch)
#   qn = l2norm(q) ; kn = l2norm(k) * (8 * q_scale * k_scale)
#   logits[c_key, i_query] = kn.T qn  (computed transposed, only the causal
#                                     triangle over x-cols and a 16-wide band
#                                     over prefix-cols)
#   P = exp(logits) * exp_bias       (bias/causal/key-mask folded into a
#                                     multiplicative exp(bias) table on host)
#   avT[d, i] = sum_j P[j, i] v[j, d], denom via an appended ones column of v
#   outT_partial = Wo_g.T @ (avT / denom)
# Host sums the two head-group partials per batch and transposes back.

import numpy as np

B, N, P, DIM, HEADS, DH = 4, 1024, 1024, 1024, 16, 64
HL = 8                 # heads per core
FL = HL * DH           # 512 local q features
J = P + N              # 2048 keys
WIND = 16              # prefix cond-window
BW = 144               # band tile width (128 cols + 16 window - 1, padded)
NEG = -1e30
CORES = list(range(8))


def _build_band_mask():
    # band[r, t] = 1 if key col (c0+r) is attended by query (c0+t):  0 <= t-r < WIND
    r = np.arange(128)[:, None]
    t = np.arange(BW)[None, :]
    return ((t - r >= 0) & (t - r < WIND)).astype(np.float32)


def _patch_tile_drain():
    """walrus in this image only encodes ~2 sem waits on a CTRL (Drain/Nop)
    instruction; Tile's exit drain attaches every outstanding sem wait to a
    single drain.  Split the waits across extra sync-engine nops."""
    import concourse.tile as tile_mod
    from concourse import mybir
    from concourse.vector_clock import ScopedClock

    if getattr(tile_mod.TileContext, "_drain_split_patch", False):
        return
    MAXW = 1

    _ENGS = {
        mybir.EngineType.PE, mybir.EngineType.Activation,
        mybir.EngineType.Pool, mybir.EngineType.DVE, mybir.EngineType.SP,
    }
    _LIMITS = {}
    _nsplit = [0]
    orig_add = tile_mod.TileContext._add_instruction

    def _add_instruction(self, inst):
        si = inst.sync_info
        lim = _LIMITS.get(inst.engine, 1)
        if (si is not None and si.on_wait and len(si.on_wait) > lim
                and inst.engine in _ENGS):
            waits = list(si.on_wait)
            keep = waits[:lim]
            rest = waits[lim:]
            inst.sync_info = mybir.SyncInfo(
                on_wait=keep, on_update=list(si.on_update or []))
            for i in range(0, len(rest), MAXW):
                _nsplit[0] += 1
                nop = mybir.InstNoOp(
                    name=f"{inst.name}-ws{_nsplit[0]}", ins=[], outs=[])
                nop.engine = inst.engine
                nop.sync_info = mybir.SyncInfo(
                    on_wait=rest[i:i + MAXW], on_update=[])
                orig_add(self, nop)
        orig_add(self, inst)

    tile_mod.TileContext._add_instruction = _add_instruction

    def _drain_and_barrier(self, tick_clock, wait_clock):
        drain_inst = self.nc.sync.drain()
        wait_clock.add_sem_waits(
            drain_inst.ins, ScopedClock({None: tick_clock.global_clock})
        )
        si = drain_inst.ins.sync_info
        waits = list(si.on_wait or []) if si is not None else []
        if len(waits) > MAXW:
            ups = list(si.on_update or []) if si is not None else []
            drain_inst.ins.sync_info = mybir.SyncInfo(on_wait=[], on_update=ups)
            for i in range(0, len(waits), MAXW):
                nop = self.nc.sync.nop(nofuse=True)
                nop.ins.sync_info = mybir.SyncInfo(
                    on_wait=waits[i:i + MAXW], on_update=[])
        self.nc.all_engine_barrier()
        assert self.sems is not None
        popped = self.nc._tile_sem_poison_stack.pop()
        assert popped is self._sem_poison
        self.nc.clear_and_free_semaphores(list(self.sems.allocated().values()))
        self.nc.all_engine_barrier()

    tile_mod.TileContext._drain_and_barrier = _drain_and_barrier
    tile_mod.TileContext._drain_split_patch = True


def _build_nc():
    import ml_dtypes
    import concourse.bass as bass
    import concourse.tile as tile
    from concourse import mybir

    _patch_tile_drain()

    f32 = mybir.dt.float32
    bf16 = mybir.dt.bfloat16
    bf = ml_dtypes.bfloat16

    nc = bass.Bass("TRN2", target_bir_lowering=False, debug=False)

    xT = nc.dram_tensor("xT", [DIM, N], bf16, kind="ExternalInput").ap()
    ctxT = nc.dram_tensor("ctxT", [DIM, P], bf16, kind="ExternalInput").ap()
    biasT = nc.dram_tensor("biasT", [HL, N, N], bf16, kind="ExternalInput").ap()
    wc = nc.dram_tensor("wc", [DIM, FL], bf16, kind="ExternalInput").ap()
    wkv = nc.dram_tensor("wkv", [DIM, 2 * DH], bf16, kind="ExternalInput").ap()
    wo = nc.dram_tensor("wo", [FL, DIM], bf16, kind="ExternalInput").ap()
    sdk = nc.dram_tensor("sdk", [DH, 1], f32, kind="ExternalInput").ap()
    outT = nc.dram_tensor("outT", [DIM, N], f32, kind="ExternalOutput").ap()

    bandm_dram = nc.inline_tensor(_build_band_mask().astype(bf), "bandm").ap()
    idup_np = (np.arange(128)[:, None] % 64 == np.arange(64)[None, :])
    idup_dram = nc.inline_tensor(idup_np.astype(bf), "idup").ap()
    # col 0 sums partitions 0-63, col 64 sums partitions 64-127 (keeps the
    # per-head sumsq rows at 32-aligned partitions 0 and 64)
    ind2_np = np.zeros((128, 128))
    ind2_np[:64, 0] = 1.0
    ind2_np[64:, 64] = 1.0
    ind2_dram = nc.inline_tensor(ind2_np.astype(bf), "ind2").ap()

    Exp = mybir.ActivationFunctionType.Exp
    Ln = mybir.ActivationFunctionType.Ln

    with tile.TileContext(nc) as tc, \
            tc.tile_pool(name="big", bufs=1) as big, \
            tc.tile_pool(name="cst", bufs=1) as cst, \
            tc.tile_pool(name="ptx", bufs=12) as ptxp, \
            tc.tile_pool(name="ptb", bufs=8) as ptbp, \
            tc.tile_pool(name="bia", bufs=3) as biap, \
            tc.tile_pool(name="sq", bufs=4) as sqp, \
            tc.tile_pool(name="small", bufs=2) as smp, \
            tc.tile_pool(name="osb", bufs=3) as osbp, \
            tc.tile_pool(name="drs", bufs=4, space="DRAM") as drsp, \
            tc.tile_pool(name="psA", bufs=3, space="PSUM") as psA, \
            tc.tile_pool(name="psB", bufs=2, space="PSUM") as psB:

        def bcast64(dst, src_row, tag):
            """broadcast a [1, n] SBUF row to [64, n] partitions of dst via a
            DRAM bounce (SBUF DMA sources cannot have stride-0 partitions)."""
            n = src_row.shape[-1]
            dt = drsp.tile([1, n], f32, tag=tag)
            nc.scalar.dma_start(out=dt[:], in_=src_row)
            nc.scalar.dma_start(out=dst, in_=dt[0:1, :].to_broadcast((64, n)))

        # ---- phase A: load everything (kv-path inputs first, loads split
        # across chunks + engines so they land on many DMA queues) ----
        wkv_sb = big.tile([128, 8, 2 * DH], bf16, tag="wkv")
        nc.sync.dma_start(wkv_sb[:], wkv.rearrange("(kt p) f -> p kt f", p=128))
        ctxT_sb = big.tile([128, 8, P], bf16, tag="ctxT")
        ctxr = ctxT.rearrange("(kt p) n -> p kt n", p=128)
        xT_sb = big.tile([128, 8, N], bf16, tag="xT")
        xr = xT.rearrange("(kt p) n -> p kt n", p=128)
        for k in range(4):
            eng = (nc.sync, nc.gpsimd, nc.scalar, nc.sync)[k % 4]
            eng.dma_start(ctxT_sb[:, 2 * k:2 * k + 2, :], ctxr[:, 2 * k:2 * k + 2, :])
        for k in range(4):
            eng = (nc.gpsimd, nc.scalar, nc.sync, nc.gpsimd)[k % 4]
            eng.dma_start(xT_sb[:, 2 * k:2 * k + 2, :], xr[:, 2 * k:2 * k + 2, :])
        wc_sb = big.tile([128, 8, FL], bf16, tag="wc")
        wcr = wc.rearrange("(kt p) f -> p kt f", p=128)
        for k in range(2):
            (nc.scalar, nc.sync)[k].dma_start(
                wc_sb[:, 4 * k:4 * k + 4, :], wcr[:, 4 * k:4 * k + 4, :])
        wo_sb = big.tile([128, 4, DIM], bf16, tag="wo")
        nc.gpsimd.dma_start(wo_sb[:], wo.rearrange("(ft p) e -> p ft e", p=128))
        sdk_sb = cst.tile([DH, 1], f32, tag="sdk")
        nc.gpsimd.dma_start(sdk_sb[:], sdk)
        bandm_sb = cst.tile([128, BW], bf16, tag="bandm")
        nc.gpsimd.dma_start(bandm_sb[:], bandm_dram)
        idup_sb = cst.tile([128, 64], bf16, tag="idup")
        nc.gpsimd.dma_start(idup_sb[:], idup_dram)
        ind2_sb = cst.tile([128, 128], bf16, tag="ind2")
        nc.gpsimd.dma_start(ind2_sb[:], ind2_dram)
        eps_sb = cst.tile([128, 1], f32, tag="eps")
        nc.vector.memset(eps_sb[:], 1e-24)

        kvT_sb = big.tile([128, J], bf16, tag="kvT")      # [2d, j] raw kv
        kn_sb = big.tile([128, J], bf16, tag="kn")        # normalized k, dup'd
        va_sb = big.tile([128, 16, DH + 1], bf16, tag="va")  # v_aug, j-major
        qn_sb = big.tile([128, 4, N], bf16, tag="qn")     # normalized q
        att_sb = big.tile([128, 4, N], bf16, tag="att")   # avT/denom (features-major)

        # ---- phases B/C/D0, emission-interleaved so PE always has work:
        #   kv(prefix) -> q(ft0) -> band(pair0) -> kv(x) -> v_aug ->
        #   q(ft1) -> band(pair1) -> ... ----
        def head_view(h):
            base = (h % 2) * 64
            return (kn_sb[base:base + 64, :], qn_sb[base:base + 64, h // 2, :])

        def emit_kv(jh):
            src = ctxT_sb if jh == 0 else xT_sb
            ps = psA.tile([128, 1024], f32, tag="A", name=f"kvps{jh}")
            for half in range(2):
                for kt in range(8):
                    nc.tensor.matmul(
                        ps[:, half * 512:(half + 1) * 512],
                        lhsT=wkv_sb[:, kt, :],
                        rhs=src[:, kt, half * 512:(half + 1) * 512],
                        start=(kt == 0), stop=(kt == 7))
            nc.vector.tensor_copy(out=kvT_sb[:, jh * 1024:(jh + 1) * 1024],
                                  in_=ps[:])
            for half in range(2):
                js = slice(jh * 1024 + half * 512, jh * 1024 + half * 512 + 512)
                pss = ps[:, half * 512:(half + 1) * 512]
                sq = sqp.tile([128, 512], bf16, tag="sq", name=f"ksq{jh}{half}")
                nc.scalar.activation(sq[0:64, :], pss[0:64, :],
                                     mybir.ActivationFunctionType.Square)
                ssq = psB.tile([128, 512], f32, tag="B", name=f"kssq{jh}{half}")
                nc.tensor.matmul(ssq[0:1, :], lhsT=ind2_sb[0:64, 0:1],
                                 rhs=sq[0:64, :], start=True, stop=True)
                rk = smp.tile([2, 512], f32, tag="rk", name=f"krk{jh}{half}")
                nc.scalar.activation(rk[0:1, :], ssq[0:1, :], Ln,
                                     bias=eps_sb[0:1])
                rkr = smp.tile([2, 512], f32, tag="rkr", name=f"krkr{jh}{half}")
                nc.scalar.activation(rkr[0:1, :], rk[0:1, :], Exp, scale=-0.5)
                rkb = smp.tile([64, 512], f32, tag="rkb", name=f"krkb{jh}{half}")
                bcast64(rkb[:], rkr[0:1, :], "drk")
                nc.vector.tensor_mul(kn_sb[0:64, js], kvT_sb[0:64, js], rkb[:])
                nc.vector.tensor_scalar_mul(kn_sb[0:64, js], kn_sb[0:64, js],
                                            sdk_sb[:])
            # duplicate this half of kn into partitions 64-127 (odd heads)
            nc.gpsimd.dma_start(
                out=kn_sb[64:128, jh * 1024:(jh + 1) * 1024],
                in_=kn_sb[0:64, jh * 1024:(jh + 1) * 1024])

        def emit_q(ft):
            ps = psA.tile([128, 1024], f32, tag="A", name=f"qps{ft}")
            for half in range(2):
                for kt in range(8):
                    nc.tensor.matmul(
                        ps[:, half * 512:(half + 1) * 512],
                        lhsT=wc_sb[:, kt, ft * 128:(ft + 1) * 128],
                        rhs=xT_sb[:, kt, half * 512:(half + 1) * 512],
                        start=(kt == 0), stop=(kt == 7))
            for half in range(2):
                qs = slice(half * 512, (half + 1) * 512)
                pss = ps[:, qs]
                sq = sqp.tile([128, 512], bf16, tag="sq", name=f"qsq{ft}{half}")
                nc.scalar.activation(sq[:], pss,
                                     mybir.ActivationFunctionType.Square)
                ssq = psB.tile([128, 512], f32, tag="B", name=f"qssq{ft}{half}")
                nc.tensor.matmul(ssq[:], lhsT=ind2_sb[:], rhs=sq[:],
                                 start=True, stop=True)
                rq = smp.tile([128, 512], f32, tag="rk", name=f"qrq{ft}{half}")
                nc.scalar.activation(rq[:], ssq[:], Ln, bias=eps_sb[:])
                rqr = smp.tile([128, 512], f32, tag="rkr", name=f"qrqr{ft}{half}")
                nc.scalar.activation(rqr[:], rq[:], Exp, scale=-0.5)
                rqb = smp.tile([128, 512], f32, tag="rqb", name=f"qrqb{ft}{half}")
                bcast64(rqb[0:64, :], rqr[0:1, :], "drq0")
                bcast64(rqb[64:128, :], rqr[64:65, :], "drq1")
                nc.vector.tensor_mul(qn_sb[:, ft, qs], pss, rqb[:])

        ptbs = []
        for h in range(HL):
            ptbs.append(ptbp.tile([128, 8, BW], bf16, tag="ptb",
                                  name=f"ptb{h}"))

        def emit_band(hp):
            # even head on partitions 0-63, odd on 64-127 (kn_dup): the
            # interleaved matmuls run on disjoint PE row groups
            for grp in range(2):
                bpss = [psA.tile([128, 1024], f32, tag="A",
                                 name=f"bps{hp}_{grp}_{k}") for k in range(2)]
                for i in range(4):
                    ct = grp * 4 + i
                    c0 = 128 * ct
                    qw = min(BW, N - c0)
                    for pr in range(2):
                        kh, qh = head_view(2 * hp + pr)
                        nc.tensor.matmul(
                            bpss[pr][:, i * 256:i * 256 + qw],
                            lhsT=kh[:, c0:c0 + 128],
                            rhs=qh[:, c0:c0 + qw],
                            start=True, stop=True)
                for pr in range(2):
                    ptb = ptbs[2 * hp + pr]
                    bview = bpss[pr][:].rearrange(
                        "p (i x) -> p i x", x=256)[:, :, 0:BW]
                    nc.scalar.activation(ptb[:, grp * 4:(grp + 1) * 4, :],
                                         bview, Exp)
                    nc.vector.tensor_mul(
                        ptb[:, grp * 4:(grp + 1) * 4, :],
                        ptb[:, grp * 4:(grp + 1) * 4, :],
                        bandm_sb[:, None, :].to_broadcast((128, 4, BW)))

        emit_kv(0)
        emit_kv(1)
        # v transposes to j-major, build v_aug
        nc.vector.memset(va_sb[:, :, DH:DH + 1], 1.0)
        vt = psB.tile([128, 1024], bf16, tag="B")
        for jt in range(16):
            nc.tensor.transpose(
                vt[:, jt * 64:(jt + 1) * 64],
                kvT_sb[64:128, jt * 128:(jt + 1) * 128],
                idup_sb[64:128, :])
        nc.vector.tensor_copy(out=va_sb[:, :, 0:DH],
                              in_=vt[:].rearrange("p (t d) -> p t d", d=64))
        for ft in range(4):
            emit_q(ft)
        for ft in range(4):
            emit_band(ft)

        # ---- phase D1/D2 + E: per query-chunk attention, then out-proj ----
        def emit_sims(h, qc):
            """sim matmuls + exp + bias-mult for one head/chunk; returns PT."""
            Q0 = qc * 512
            nct = 4 * (qc + 1)
            kh, qh = head_view(h)
            # bias fetch for all col-tiles of this chunk, split in 2-ct DMAs
            # so transfers spread over several queues; masked regions are
            # exactly 0, which also zeroes PT garbage
            bt = biap.tile([128, 8, 512], bf16, tag="bias")
            btr = biasT[h].rearrange("(ct p) i -> p ct i", p=128)
            for pg in range((nct + 1) // 2):
                nc.gpsimd.dma_start(
                    out=bt[:, 2 * pg:2 * pg + 2, :],
                    in_=btr[:, 2 * pg:2 * pg + 2, Q0:Q0 + 512])
            ptxs = []
            for pg in range((nct + 1) // 2):
                sps = psA.tile([128, 1024], f32, tag="A")
                ptx = ptxp.tile([128, 1024], bf16, tag="ptx")
                for i in range(2):
                    ct = pg * 2 + i
                    if ct >= nct:
                        continue
                    c0 = 128 * ct
                    off = max(0, c0 - Q0)
                    nc.tensor.matmul(
                        sps[:, i * 512 + off:(i + 1) * 512],
                        lhsT=kh[:, P + c0:P + c0 + 128],
                        rhs=qh[:, Q0 + off:Q0 + 512],
                        start=True, stop=True)
                nc.scalar.activation(ptx[:], sps[:], Exp)
                nc.vector.tensor_mul(
                    ptx[:],
                    ptx[:],
                    bt[:, pg * 2:pg * 2 + 2, :].rearrange("p a b -> p (a b)"))
                ptxs.append(ptx)
            return ptxs

        def emit_av(h, qc, ptxs):
            """AV accumulation + softmax normalize into att_sb."""
            Q0 = qc * 512
            nct = 4 * (qc + 1)
            base = (h % 2) * 64
            ft = h // 2
            ptb = ptbs[h]
            aps = psB.tile([128, 512], f32, tag="B")
            av_mms = []
            for ct in range(nct):
                c0 = 128 * ct
                off = max(0, c0 - Q0)
                av_mms.append((
                    aps[0:DH + 1, off:512], va_sb[:, 8 + ct, :],
                    ptxs[ct // 2][:, (ct % 2) * 512 + off:(ct % 2 + 1) * 512]))
            for ct in range(8):
                c0 = 128 * ct
                qw = min(BW, N - c0)
                lo = max(c0, Q0)
                hi = min(c0 + qw, Q0 + 512)
                if lo >= hi:
                    continue
                av_mms.append((
                    aps[0:DH + 1, lo - Q0:hi - Q0], va_sb[:, ct, :],
                    ptb[:, ct, lo - c0:hi - c0]))
            for i, (o, l, r) in enumerate(av_mms):
                nc.tensor.matmul(o, lhsT=l, rhs=r, start=(i == 0),
                                 stop=(i == len(av_mms) - 1))
            # normalize: 1/denom = exp(-ln(denom)), broadcast over rows
            rel = smp.tile([1, 512], f32, tag="rel")
            nc.scalar.activation(rel[:], aps[DH:DH + 1, :], Ln)
            rec = smp.tile([1, 512], f32, tag="rec")
            nc.scalar.activation(rec[:], rel[:], Exp, scale=-1.0)
            recb = smp.tile([64, 512], f32, tag="recb")
            bcast64(recb[:], rec[:], "drec")
            nc.vector.tensor_mul(
                att_sb[base:base + 64, ft, Q0:Q0 + 512],
                aps[0:64, :],
                recb[:])

        for qc in range(2):
            # per head-pair: both heads' sims, then both heads' avs
            for hp in range(4):
                pts = [emit_sims(2 * hp + pr, qc) for pr in range(2)]
                for pr in range(2):
                    emit_av(2 * hp + pr, qc, pts[pr])

            # out-proj for this chunk
            for et in range(8):
                ops = psB.tile([128, 512], f32, tag="B")
                for ftile in range(4):
                    nc.tensor.matmul(
                        ops[:],
                        lhsT=wo_sb[:, ftile, et * 128:(et + 1) * 128],
                        rhs=att_sb[:, ftile, qc * 512:(qc + 1) * 512],
                        start=(ftile == 0), stop=(ftile == 3))
                o_sb = osbp.tile([128, 512], f32, tag="osb")
                nc.vector.tensor_copy(out=o_sb[:], in_=ops[:])
                nc.gpsimd.dma_start(
                    out=outT[et * 128:(et + 1) * 128, qc * 512:(qc + 1) * 512],
                    in_=o_sb[:])

    return nc


_NC = None


def _get_nc():
    global _NC
    if _NC is None:
        _NC = _build_nc()
    return _NC


def _prep_in_maps(x, prefix_context, attn_bias, gamma, Wq, Wkv, q_scale,
                  k_scale, Wo, mask):
    import ml_dtypes
    bf = ml_dtypes.bfloat16

    x = np.asarray(x, np.float32)
    prefix_context = np.asarray(prefix_context, np.float32)
    attn_bias = np.asarray(attn_bias, np.float32)
    gamma = np.asarray(gamma, np.float32)
    Wq = np.asarray(Wq, np.float32)
    Wkv = np.asarray(Wkv, np.float32)
    q_scale = np.asarray(q_scale, np.float32)
    k_scale = np.asarray(k_scale, np.float32)
    Wo = np.asarray(Wo, np.float32)
    mask = np.asarray(mask)

    tril = np.triu(np.ones((N, N), np.float32))  # keep key c <= query i ([c, i])
    sdk_np = (8.0 * q_scale * k_scale).astype(np.float32).reshape(DH, 1)
    wkvT = np.ascontiguousarray(Wkv.T).astype(bf)

    in_maps = []
    for c in CORES:
        b, g = c // 2, c % 2
        hs = slice(g * HL, (g + 1) * HL)
        # exp(bias) with causal kill and key-mask folded in, [h, c, i] layout
        eb = np.exp(attn_bias[hs].transpose(0, 2, 1)) * tril[None]
        maskf = mask[b].astype(np.float32)
        if not maskf.all():
            eb = eb * maskf[None, :, None]
        Wg = Wq[g * FL:(g + 1) * FL] * gamma[None, :]
        s = Wg.sum(axis=1)
        wcT = Wg.T - s[None, :] / DIM
        wog = Wo[:, g * FL:(g + 1) * FL]
        in_maps.append(dict(
            xT=np.ascontiguousarray(x[b].T).astype(bf),
            ctxT=np.ascontiguousarray(prefix_context[b].T).astype(bf),
            biasT=np.ascontiguousarray(eb).astype(bf),
            wc=np.ascontiguousarray(wcT).astype(bf),
            wkv=wkvT,
            wo=np.ascontiguousarray(wog.T).astype(bf),
            sdk=sdk_np,
        ))
    return in_maps


def kernel(**inputs):
    from concourse.bass_utils import run_bass_kernel_spmd

    nc = _get_nc()
    in_maps = _prep_in_maps(**inputs)
    res = run_bass_kernel_spmd(nc, in_maps, CORES).results
    out = np.empty((B, N, DIM), np.float32)
    for b in range(B):
        out[b] = (np.asarray(res[2 * b]["outT"], np.float32)
                  + np.asarray(res[2 * b + 1]["outT"], np.float32)).T
    return out



# revision 3
# speedup vs baseline: 1.0574x; 1.0574x over previous
# Distributed sparse-attention kernel for Trainium2 (8 NeuronCores).
#
# Sharding: core c = (batch b = c//2, head-group g = c%2 of 8 heads).
# Per core, heads are split into two PE partition groups g2 = h%2 (even heads
# on partitions 0-63, odd on 64-127) with ft = h//2 indexing the 4 heads of a
# group.  Attention is computed per 256-query chunk, fused sim->exp->mask->AV:
#   q   = meancenter(x) @ Wc            (LN folded into Wc on host)
#   kv  = [prefix; x] @ Wkv.T           (MQA single head)
#   qn  = q * (8*q_scale*k_scale) * rsqrt(sumsq(q))   (per-query bcast via a
#         block-ones reduce matmul so no partition-broadcast is needed)
#   kn  = raw k; 1/||k|| folded into the Exp activation's per-partition scale
#   P   = exp(kn.T qn * rk) * exp_bias  (bias/causal folded into a
#         multiplicative exp(bias) table, host-packed per 128-key tile)
#   AV  uses va = [v | ones] as stationary so PSUM rows 64-127 hold the
#         softmax denominator pre-broadcast; normalize = reciprocal + mult
#   out = Wo_g.T @ att, summed across the two head-group cores on host.

import numpy as np

B, N, P, DIM, HEADS, DH = 4, 1024, 1024, 1024, 16, 64
HL = 8                 # heads per core
FL = HL * DH           # 512 local q features
J = P + N              # 2048 keys
WIND = 16              # prefix cond-window
QW = 256               # query chunk
NQC = N // QW          # 4 query chunks
CORES = list(range(8))


def _x_units(qch):
    """x-region key tiles for query chunk qch: (ct, qlo, qhi)."""
    q0 = qch * QW
    return [(ct, max(q0, 128 * ct), q0 + QW) for ct in range(2 * qch + 2)]


def _band_units(qch):
    """prefix band tiles: (jt, qlo, qhi, maskkind)."""
    q0 = qch * QW
    out = []
    if qch > 0:
        out.append((2 * qch - 1, q0, q0 + WIND, "corner"))
    out.append((2 * qch, q0, q0 + 144, "main"))
    out.append((2 * qch + 1, q0 + 128, q0 + QW, "third"))
    return out


def _bias_layout():
    """Offsets of host-packed bias tiles keyed (qch, g2, ct); emission order
    must match _prep_in_maps' packing order exactly."""
    offs = {}
    off = 0
    for qch in range(NQC):
        for g2 in range(2):
            for ct, qlo, qhi in _x_units(qch):
                w = qhi - qlo
                offs[(qch, g2, ct)] = (off, w)
                off += 128 * 4 * w
    return offs, off


BIAS_OFFS, BIAS_TOTAL = _bias_layout()


def _masks():
    r = np.arange(128)[:, None]
    t = np.arange(144)[None, :]
    main = ((t - r >= 0) & (t - r < WIND)).astype(np.float32)      # [128,144]
    tc = np.arange(WIND)[None, :]
    corner = (r - tc >= 128 - WIND + 1).astype(np.float32)         # [128,16]
    return main, corner


def _patch_tile_drain():
    """walrus in this image only encodes ~2 sem waits on a CTRL (Drain/Nop)
    instruction; Tile's exit drain attaches every outstanding sem wait to a
    single drain.  Split the waits across extra sync-engine nops."""
    import concourse.tile as tile_mod
    from concourse import mybir
    from concourse.vector_clock import ScopedClock

    if getattr(tile_mod.TileContext, "_drain_split_patch", False):
        return
    MAXW = 1

    _ENGS = {
        mybir.EngineType.PE, mybir.EngineType.Activation,
        mybir.EngineType.Pool, mybir.EngineType.DVE, mybir.EngineType.SP,
    }
    _LIMITS = {}
    _nsplit = [0]
    orig_add = tile_mod.TileContext._add_instruction

    def _add_instruction(self, inst):
        si = inst.sync_info
        lim = _LIMITS.get(inst.engine, 1)
        if (si is not None and si.on_wait and len(si.on_wait) > lim
                and inst.engine in _ENGS):
            waits = list(si.on_wait)
            keep = waits[:lim]
            rest = waits[lim:]
            inst.sync_info = mybir.SyncInfo(
                on_wait=keep, on_update=list(si.on_update or []))
            for i in range(0, len(rest), MAXW):
                _nsplit[0] += 1
                nop = mybir.InstNoOp(
                    name=f"{inst.name}-ws{_nsplit[0]}", ins=[], outs=[])
                nop.engine = inst.engine
                nop.sync_info = mybir.SyncInfo(
                    on_wait=rest[i:i + MAXW], on_update=[])
                orig_add(self, nop)
        orig_add(self, inst)

    tile_mod.TileContext._add_instruction = _add_instruction

    def _drain_and_barrier(self, tick_clock, wait_clock):
        drain_inst = self.nc.sync.drain()
        wait_clock.add_sem_waits(
            drain_inst.ins, ScopedClock({None: tick_clock.global_clock})
        )
        si = drain_inst.ins.sync_info
        waits = list(si.on_wait or []) if si is not None else []
        if len(waits) > MAXW:
            ups = list(si.on_update or []) if si is not None else []
            drain_inst.ins.sync_info = mybir.SyncInfo(on_wait=[], on_update=ups)
            for i in range(0, len(waits), MAXW):
                nop = self.nc.sync.nop(nofuse=True)
                nop.ins.sync_info = mybir.SyncInfo(
                    on_wait=waits[i:i + MAXW], on_update=[])
        self.nc.all_engine_barrier()
        assert self.sems is not None
        popped = self.nc._tile_sem_poison_stack.pop()
        assert popped is self._sem_poison
        self.nc.clear_and_free_semaphores(list(self.sems.allocated().values()))
        self.nc.all_engine_barrier()

    tile_mod.TileContext._drain_and_barrier = _drain_and_barrier
    tile_mod.TileContext._drain_split_patch = True


def _build_nc():
    import ml_dtypes
    import concourse.bass as bass
    import concourse.tile as tile
    from concourse import mybir
    from concourse.alu_op_type import AluOpType

    _patch_tile_drain()

    f32 = mybir.dt.float32
    bf16 = mybir.dt.bfloat16
    bf = ml_dtypes.bfloat16

    nc = bass.Bass("TRN2", target_bir_lowering=False, debug=False)

    xT = nc.dram_tensor("xT", [DIM, N], bf16, kind="ExternalInput").ap()
    ctxT = nc.dram_tensor("ctxT", [DIM, P], bf16, kind="ExternalInput").ap()
    biasF = nc.dram_tensor("biasF", [BIAS_TOTAL], bf16,
                           kind="ExternalInput").ap()
    wc = nc.dram_tensor("wc", [DIM, FL], bf16, kind="ExternalInput").ap()
    wkv = nc.dram_tensor("wkv", [DIM, 2 * DH], bf16, kind="ExternalInput").ap()
    wo = nc.dram_tensor("wo", [FL, DIM], bf16, kind="ExternalInput").ap()
    sdkc = nc.dram_tensor("sdkc", [128, 1], f32, kind="ExternalInput").ap()
    outT = nc.dram_tensor("outT", [DIM, N], bf16, kind="ExternalOutput").ap()

    mmain, mcorner = _masks()
    bandm_dram = nc.inline_tensor(mmain.astype(bf), "bandm").ap()
    corner_dram = nc.inline_tensor(mcorner.astype(bf), "cornm").ap()
    idup_np = (np.arange(128)[:, None] % 64 == np.arange(64)[None, :])
    idup_dram = nc.inline_tensor(idup_np.astype(bf), "idup").ap()
    # block-ones: col m sums partitions 64*(m//64) .. +64 -> row-broadcast ssq
    indb_np = (np.arange(128)[:, None] // 64
               == np.arange(128)[None, :] // 64)
    indb_dram = nc.inline_tensor(indb_np.astype(bf), "indb").ap()

    Exp = mybir.ActivationFunctionType.Exp
    Ln = mybir.ActivationFunctionType.Ln
    Sq = mybir.ActivationFunctionType.Square
    Cp = mybir.ActivationFunctionType.Copy

    with tile.TileContext(nc) as tc, \
            tc.tile_pool(name="big", bufs=1) as big, \
            tc.tile_pool(name="cst", bufs=1) as cst, \
            tc.tile_pool(name="sqp", bufs=2) as sqp, \
            tc.tile_pool(name="lnp", bufs=2) as lnp, \
            tc.tile_pool(name="bia", bufs=10) as biap, \
            tc.tile_pool(name="ptx", bufs=6) as ptxp, \
            tc.tile_pool(name="rcb", bufs=4) as rcbp, \
            tc.tile_pool(name="osb", bufs=3) as osbp, \
            tc.tile_pool(name="psS", bufs=2, space="PSUM") as psS, \
            tc.tile_pool(name="psA", bufs=1, space="PSUM") as psA:

        # ---- loads ----
        xr = xT.rearrange("(kt p) n -> p kt n", p=128)
        x_sb = big.tile([128, 8, N], bf16, tag="xT")
        for h in range(2):
            for k in range(4):
                nc.sync.dma_start(
                    x_sb[:, 2 * k:2 * k + 2, h * 512:(h + 1) * 512],
                    xr[:, 2 * k:2 * k + 2, h * 512:(h + 1) * 512])
        ctxr = ctxT.rearrange("(kt p) n -> p kt n", p=128)
        ctx_sb = big.tile([128, 8, P], bf16, tag="ctxT")
        for h in range(2):
            for k in range(4):
                nc.sync.dma_start(
                    ctx_sb[:, 2 * k:2 * k + 2, h * 512:(h + 1) * 512],
                    ctxr[:, 2 * k:2 * k + 2, h * 512:(h + 1) * 512])

        wcr = wc.rearrange("(kt p) f -> p kt f", p=128)
        wc_sb = big.tile([128, 8, FL], bf16, tag="wc")
        for ft in range(4):
            nc.scalar.dma_start(wc_sb[:, :, ft * 128:(ft + 1) * 128],
                                wcr[:, :, ft * 128:(ft + 1) * 128])
        wkv_sb = big.tile([128, 8, 2 * DH], bf16, tag="wkv")
        nc.scalar.dma_start(wkv_sb[:], wkv.rearrange("(kt p) f -> p kt f",
                                                     p=128))
        idup_sb = cst.tile([128, 64], bf16, tag="idup")
        nc.scalar.dma_start(idup_sb[:], idup_dram)
        indb_sb = cst.tile([128, 128], bf16, tag="indb")
        nc.scalar.dma_start(indb_sb[:], indb_dram)
        sdk_sb = cst.tile([128, 1], f32, tag="sdk")
        nc.scalar.dma_start(sdk_sb[:], sdkc)
        bandm_sb = cst.tile([128, 144], bf16, tag="bandm")
        nc.scalar.dma_start(bandm_sb[:], bandm_dram)
        corner_sb = cst.tile([128, WIND], bf16, tag="cornm")
        nc.scalar.dma_start(corner_sb[:], corner_dram)
        wo_sb = big.tile([128, 4, DIM], bf16, tag="wo")
        nc.scalar.dma_start(wo_sb[:], wo.rearrange("(ft p) e -> p ft e",
                                                   p=128))

        eps_sb = cst.tile([128, 1], f32, tag="eps")
        nc.vector.memset(eps_sb[:], 1e-24)
        garb_sb = cst.tile([128, 512], bf16, tag="garb")
        nc.vector.memset(garb_sb[:], 0.125)

        kvT_sb = big.tile([128, J], bf16, tag="kvT")   # k rows 0-63, v 64-127
        kdup_sb = big.tile([128, J], bf16, tag="kdup")  # k dup'd on 64-127
        qn_sb = big.tile([128, 4, N], bf16, tag="qn")
        att_sb = big.tile([128, 4, N], bf16, tag="att")
        va_sb = big.tile([128, 16, 128], bf16, tag="va")  # [v | ones]
        nc.vector.memset(va_sb[:, :, DH:128], 1.0)
        rkT_sb = cst.tile([128, 16], f32, tag="rkT")
        ssqT_sb = cst.tile([128, 16], f32, tag="ssqT")

        # ---- PE warmup during loads ----
        for i in range(16):
            wps = psS.tile([128, 1024], f32, tag="S", name=f"warm{i}")
            nc.tensor.matmul(wps[:, 0:512], lhsT=garb_sb[:, 0:128],
                             rhs=garb_sb[:], start=True, stop=True)

        # ---- q projection + normalize (deferred one ft for overlap) ----
        def emit_q_mm(ft):
            ps = psS.tile([128, 1024], f32, tag="S", name=f"qps{ft}")
            for kt in range(8):
                for h in range(2):
                    nc.tensor.matmul(
                        ps[:, h * 512:(h + 1) * 512],
                        lhsT=wc_sb[:, kt, ft * 128:(ft + 1) * 128],
                        rhs=x_sb[:, kt, h * 512:(h + 1) * 512],
                        start=(kt == 0), stop=(kt == 7))
            return ps

        def emit_q_norm(ft, ps):
            sq = sqp.tile([128, 1024], bf16, tag="sq", name=f"qsq{ft}")
            nc.scalar.activation(sq[:], ps[:], Sq)
            ssq = psA.tile([128, 2048], f32, tag="av", name=f"qssq{ft}")
            for h in range(2):
                nc.tensor.matmul(ssq[:, h * 512:(h + 1) * 512],
                                 lhsT=indb_sb[:],
                                 rhs=sq[:, h * 512:(h + 1) * 512],
                                 start=True, stop=True)
            lnq = lnp.tile([128, 1024], f32, tag="ln", name=f"qln{ft}")
            nc.scalar.activation(lnq[:], ssq[:, 0:1024], Ln, bias=eps_sb[:])
            rqb = lnp.tile([128, 1024], f32, tag="rqb", name=f"qrq{ft}")
            nc.scalar.activation(rqb[:], lnq[:], Exp, scale=-0.5)
            # qn = (q * sdk) * rsqrt(ssq)
            nc.vector.scalar_tensor_tensor(
                out=qn_sb[:, ft, :], in0=ps[:], scalar=sdk_sb[:], in1=rqb[:],
                op0=AluOpType.mult, op1=AluOpType.mult)

        def emit_kv(jh):
            src = ctx_sb if jh == 0 else x_sb
            ps = psS.tile([128, 1024], f32, tag="S", name=f"kvps{jh}")
            for kt in range(8):
                for h in range(2):
                    nc.tensor.matmul(
                        ps[:, h * 512:(h + 1) * 512],
                        lhsT=wkv_sb[:, kt, :],
                        rhs=src[:, kt, h * 512:(h + 1) * 512],
                        start=(kt == 0), stop=(kt == 7))
            js = slice(jh * 1024, (jh + 1) * 1024)
            nc.vector.tensor_copy(out=kvT_sb[:, js], in_=ps[:])
            nc.scalar.activation(kdup_sb[64:128, js], ps[0:64, :], Cp)

        def emit_ktr(jh):
            """transpose k/v tiles of half jh; build va, rkT."""
            js0 = jh * 1024
            ktps = psA.tile([128, 2048], f32, tag="av", name=f"ktp{jh}")
            ktv = ktps[:, 0:256].bitcast(bf16)       # [128, 512] bf16
            for i in range(8):
                nc.tensor.transpose(
                    ktv[:, i * 64:(i + 1) * 64],
                    kvT_sb[0:64, js0 + i * 128:js0 + (i + 1) * 128],
                    idup_sb[0:64, :])
            sqk = sqp.tile([128, 512], bf16, tag="sqk", name=f"sqk{jh}")
            nc.scalar.activation(sqk[:], ktv[:], Sq)
            nc.vector.tensor_reduce(
                ssqT_sb[:, jh * 8:(jh + 1) * 8],
                sqk[:].rearrange("p (t d) -> p t d", d=64),
                mybir.AxisListType.X, AluOpType.add)
            lnk = lnp.tile([128, 16], f32, tag="lnk", name=f"lnk{jh}")
            nc.scalar.activation(lnk[:, 0:8], ssqT_sb[:, jh * 8:(jh + 1) * 8],
                                 Ln, bias=eps_sb[:])
            nc.scalar.activation(rkT_sb[:, jh * 8:(jh + 1) * 8], lnk[:, 0:8],
                                 Exp, scale=-0.5)
            vtps = psA.tile([128, 2048], f32, tag="av", name=f"vtp{jh}")
            vtv = vtps[:, 0:256].bitcast(bf16)
            for i in range(8):
                nc.tensor.transpose(
                    vtv[:, i * 64:(i + 1) * 64],
                    kvT_sb[64:128, js0 + i * 128:js0 + (i + 1) * 128],
                    idup_sb[64:128, :])
            nc.vector.tensor_copy(
                out=va_sb[:, jh * 8:(jh + 1) * 8, 0:DH],
                in_=vtv[:].rearrange("p (t d) -> p t d", d=64))

        qpss = {}
        qpss[0] = emit_q_mm(0)
        qpss[1] = emit_q_mm(1)
        emit_q_norm(0, qpss[0])
        emit_kv(0)
        emit_q_norm(1, qpss[1])
        qpss[2] = emit_q_mm(2)
        emit_ktr(0)
        qpss[3] = emit_q_mm(3)
        emit_q_norm(2, qpss[2])
        emit_kv(1)
        emit_q_norm(3, qpss[3])
        emit_ktr(1)

        # ---- fused attention per query chunk ----
        def emit_sim(qch, g2, jt, qlo, qhi, kind, ct=None):
            """sim matmuls + exp(scale=1/||k||) + bias/mask mult -> ptx."""
            w = qhi - qlo
            base = 64 * g2
            lhs = kvT_sb if g2 == 0 else kdup_sb
            ps = psS.tile([128, 1024], f32, tag="S",
                          name=f"sps{qch}{g2}{jt}{kind}")
            for ft in range(4):
                nc.tensor.matmul(
                    ps[:, ft * 256:ft * 256 + w],
                    lhsT=lhs[base:base + 64, jt * 128:(jt + 1) * 128],
                    rhs=qn_sb[base:base + 64, ft, qlo:qhi],
                    start=True, stop=True)
            pt = ptxp.tile([128, 4, 256], bf16, tag="ptx",
                           name=f"ptx{qch}{g2}{jt}{kind}")
            psv = ps[:].rearrange("p (f x) -> p f x", x=256)[:, :, 0:w]
            nc.scalar.activation(pt[:, :, 0:w], psv, Exp,
                                 scale=rkT_sb[:, jt:jt + 1])
            if kind == "x":
                off, bw = BIAS_OFFS[(qch, g2, ct)]
                assert bw == w
                bt = biap.tile([128, 4, 256], bf16, tag="bias",
                               name=f"bt{qch}{g2}{jt}")
                nc.sync.dma_start(
                    bt[:, :, 0:w],
                    biasF[off:off + 128 * 4 * w].rearrange(
                        "(p f x) -> p f x", p=128, f=4))
                nc.vector.tensor_mul(pt[:, :, 0:w], pt[:, :, 0:w],
                                     bt[:, :, 0:w])
            else:
                msk = (corner_sb if kind == "corner" else bandm_sb)[:, 0:w]
                nc.vector.tensor_mul(
                    pt[:, :, 0:w], pt[:, :, 0:w],
                    msk[:, None, :].to_broadcast((128, 4, w)))
            return pt

        def emit_av(avps, qch, g2, jt, qlo, qhi, pt, start, stop):
            q0 = qch * QW
            w = qhi - qlo
            for ft in range(4):
                c0 = 512 * ft + 256 * g2 + (qlo - q0)
                nc.tensor.matmul(avps[:, c0:c0 + w],
                                 lhsT=va_sb[:, jt, :],
                                 rhs=pt[:, ft, 0:w],
                                 start=start, stop=stop)

        def emit_attnorm(avps, qch):
            q0 = qch * QW
            avv = avps[:].rearrange("p (b x) -> p b x", x=512)
            for g2 in range(2):
                rb = rcbp.tile([64, 4, 256], f32, tag="rcb",
                               name=f"rcb{qch}{g2}")
                sl = slice(256 * g2, 256 * g2 + 256)
                nc.vector.reciprocal(rb[:], avv[64:128, :, sl])
                nc.vector.tensor_mul(
                    att_sb[64 * g2:64 * g2 + 64, :, q0:q0 + QW],
                    avv[0:64, :, sl], rb[:])

        def emit_outproj(qc):
            for et in range(8):
                ops = psS.tile([128, 1024], f32, tag="S", name=f"op{qc}{et}")
                for ftile in range(4):
                    nc.tensor.matmul(
                        ops[:, 0:512],
                        lhsT=wo_sb[:, ftile, et * 128:(et + 1) * 128],
                        rhs=att_sb[:, ftile, qc * 512:(qc + 1) * 512],
                        start=(ftile == 0), stop=(ftile == 3))
                o = osbp.tile([128, 512], bf16, tag="osb", name=f"o{qc}{et}")
                if et % 2 == 0:
                    nc.vector.tensor_copy(out=o[:], in_=ops[:, 0:512])
                else:
                    nc.scalar.activation(o[:], ops[:, 0:512], Cp)
                nc.gpsimd.dma_start(
                    out=outT[et * 128:(et + 1) * 128,
                             qc * 512:(qc + 1) * 512],
                    in_=o[:])

        for qch in range(NQC):
            avps = psA.tile([128, 2048], f32, tag="av", name=f"av{qch}")
            for g2 in range(2):
                xs = _x_units(qch)
                units = ([("x",) + xs[0]]
                         + [("b", jt, lo, hi, kk)
                            for jt, lo, hi, kk in _band_units(qch)]
                         + [("x",) + u for u in xs[1:]])
                pend = None
                for i, u in enumerate(units):
                    if u[0] == "x":
                        _, ct, qlo, qhi = u
                        jt, kind = 8 + ct, "x"
                        pt = emit_sim(qch, g2, jt, qlo, qhi, kind, ct=ct)
                    else:
                        _, jt, qlo, qhi, kind = u
                        pt = emit_sim(qch, g2, jt, qlo, qhi, kind)
                    if pend is not None:
                        emit_av(avps, qch, g2, *pend, start=(i == 1),
                                stop=False)
                    pend = (jt, qlo, qhi, pt)
                emit_av(avps, qch, g2, *pend, start=False, stop=True)
            emit_attnorm(avps, qch)
            if qch == 1:
                emit_outproj(0)
            if qch == 3:
                emit_outproj(1)

    return nc


_NC = None


def _get_nc():
    global _NC
    if _NC is None:
        _NC = _build_nc()
    return _NC


def _prep_in_maps(x, prefix_context, attn_bias, gamma, Wq, Wkv, q_scale,
                  k_scale, Wo, mask):
    import ml_dtypes
    bf = ml_dtypes.bfloat16

    x = np.asarray(x, np.float32)
    prefix_context = np.asarray(prefix_context, np.float32)
    attn_bias = np.asarray(attn_bias, np.float32)
    gamma = np.asarray(gamma, np.float32)
    Wq = np.asarray(Wq, np.float32)
    Wkv = np.asarray(Wkv, np.float32)
    q_scale = np.asarray(q_scale, np.float32)
    k_scale = np.asarray(k_scale, np.float32)
    Wo = np.asarray(Wo, np.float32)
    mask = np.asarray(mask)

    tril = np.triu(np.ones((N, N), np.float32))  # keep key c <= query i [c, i]
    sdk = (8.0 * q_scale * k_scale).astype(np.float32)
    sdkc = np.tile(sdk, 2).reshape(128, 1)
    wkvT = np.ascontiguousarray(Wkv.T).astype(bf)

    in_maps = []
    for c in CORES:
        b, g = c // 2, c % 2
        hs = slice(g * HL, (g + 1) * HL)
        # exp(bias) with causal kill and key-mask folded in, [h, c, i] layout
        eb = np.exp(attn_bias[hs].transpose(0, 2, 1)) * tril[None]
        maskf = mask[b].astype(np.float32)
        if not maskf.all():
            eb = eb * maskf[None, :, None]
        eb = eb.astype(bf)
        # pack bias tiles in kernel consumption order
        bflat = np.empty(BIAS_TOTAL, bf)
        for qch in range(NQC):
            for g2 in range(2):
                heads = [2 * ft + g2 for ft in range(4)]
                for ct, qlo, qhi in _x_units(qch):
                    off, w = BIAS_OFFS[(qch, g2, ct)]
                    t = eb[heads, 128 * ct:128 * (ct + 1), qlo:qhi]
                    bflat[off:off + 128 * 4 * w] = (
                        t.transpose(1, 0, 2).ravel())
        Wg = Wq[g * FL:(g + 1) * FL] * gamma[None, :]
        s = Wg.sum(axis=1)
        wcT = Wg.T - s[None, :] / DIM
        wog = Wo[:, g * FL:(g + 1) * FL]
        in_maps.append(dict(
            xT=np.ascontiguousarray(x[b].T).astype(bf),
            ctxT=np.ascontiguousarray(prefix_context[b].T).astype(bf),
            biasF=bflat,
            wc=np.ascontiguousarray(wcT).astype(bf),
            wkv=wkvT,
            wo=np.ascontiguousarray(wog.T).astype(bf),
            sdkc=sdkc,
        ))
    return in_maps


def kernel(**inputs):
    from concourse.bass_utils import run_bass_kernel_spmd

    nc = _get_nc()
    in_maps = _prep_in_maps(**inputs)
    res = run_bass_kernel_spmd(nc, in_maps, CORES).results
    out = np.empty((B, N, DIM), np.float32)
    for b in range(B):
        out[b] = (np.asarray(res[2 * b]["outT"]).astype(np.float32)
                  + np.asarray(res[2 * b + 1]["outT"]).astype(np.float32)).T
    return out


# revision 15
# speedup vs baseline: 1.2224x; 1.1560x over previous
# Distributed sparse-attention kernel for Trainium2 (8 NeuronCores).
#
# Sharding: core c = (batch b = c//2, head-group g = c%2 of 8 heads).
# Per core, heads are split into two PE partition groups g2 = h%2 (even heads
# on partitions 0-63, odd on 64-127) with ft = h//2 indexing the 4 heads of a
# group.  Attention is computed per 256-query chunk, fused sim->exp->mask->AV:
#   q   = meancenter(x) @ Wc            (LN folded into Wc on host)
#   kv  = [prefix; x] @ Wkv.T           (MQA single head)
#   qn  = q * (8*q_scale*k_scale) * rsqrt(sumsq(q))   (per-query bcast via a
#         block-ones reduce matmul so no partition-broadcast is needed)
#   kn  = raw k; 1/||k|| folded into the Exp activation's per-partition scale
#   P   = exp(kn.T qn * rk) * exp_bias  (bias/causal folded into a
#         multiplicative exp(bias) table, host-packed per 128-key tile)
#   AV  uses va = [v | ones] as stationary so PSUM rows 64-127 hold the
#         softmax denominator pre-broadcast; normalize = reciprocal + mult
#   out = Wo_g.T @ att, summed across the two head-group cores on host.

import numpy as np

B, N, P, DIM, HEADS, DH = 4, 1024, 1024, 1024, 16, 64
HL = 8                 # heads per core
FL = HL * DH           # 512 local q features
J = P + N              # 2048 keys
WIND = 16              # prefix cond-window
QW = 256               # query chunk
NQC = N // QW          # 4 query chunks
CORES = list(range(8))


def _x_units(qch):
    """x-region key tiles for query chunk qch: (ct, qlo, qhi)."""
    q0 = qch * QW
    return [(ct, max(q0, 128 * ct), q0 + QW) for ct in range(2 * qch + 2)]


def _band_units(qch):
    """prefix band tiles: (jt, qlo, qhi, maskkind)."""
    q0 = qch * QW
    out = []
    if qch > 0:
        out.append((2 * qch - 1, q0, q0 + WIND, "corner"))
    out.append((2 * qch, q0, q0 + 144, "main"))
    out.append((2 * qch + 1, q0 + 128, q0 + QW, "third"))
    return out


def _bias_layout():
    """Offsets of host-packed bias tiles keyed (qch, g2, ct); emission order
    must match _prep_in_maps' packing order exactly."""
    offs = {}
    off = 0
    for qch in range(NQC):
        for g2 in range(2):
            for ct, qlo, qhi in _x_units(qch):
                w = qhi - qlo
                offs[(qch, g2, ct)] = (off, w)
                off += 128 * 4 * w
    return offs, off


BIAS_OFFS, BIAS_TOTAL = _bias_layout()


def _masks():
    """(q, ft)-major band masks, one per band-tile kind."""
    r = np.arange(128)[:, None]

    def rep4(m2):
        return np.repeat(m2[:, :, None], 4, axis=2).reshape(128, -1)

    t = np.arange(144)[None, :]
    main = rep4(((t - r >= 0) & (t - r < WIND)).astype(np.float32))
    tc = np.arange(WIND)[None, :]
    corner = rep4((r - tc >= 128 - WIND + 1).astype(np.float32))
    t3 = np.arange(128)[None, :]
    third = rep4(((t3 - r >= 0) & (t3 - r < WIND)).astype(np.float32))
    return main, corner, third


def _patch_tile_drain():
    """walrus in this image only encodes ~2 sem waits on a CTRL (Drain/Nop)
    instruction; Tile's exit drain attaches every outstanding sem wait to a
    single drain.  Split the waits across extra sync-engine nops."""
    import concourse.tile as tile_mod
    from concourse import mybir
    from concourse.vector_clock import ScopedClock

    if getattr(tile_mod.TileContext, "_drain_split_patch", False):
        return
    MAXW = 1

    _ENGS = {
        mybir.EngineType.PE, mybir.EngineType.Activation,
        mybir.EngineType.Pool, mybir.EngineType.DVE, mybir.EngineType.SP,
    }
    _LIMITS = {}
    _nsplit = [0]
    orig_add = tile_mod.TileContext._add_instruction

    def _add_instruction(self, inst):
        si = inst.sync_info
        lim = _LIMITS.get(inst.engine, 1)
        if (si is not None and si.on_wait and len(si.on_wait) > lim
                and inst.engine in _ENGS):
            waits = list(si.on_wait)
            keep = waits[:lim]
            rest = waits[lim:]
            inst.sync_info = mybir.SyncInfo(
                on_wait=keep, on_update=list(si.on_update or []))
            for i in range(0, len(rest), MAXW):
                _nsplit[0] += 1
                nop = mybir.InstNoOp(
                    name=f"{inst.name}-ws{_nsplit[0]}", ins=[], outs=[])
                nop.engine = inst.engine
                nop.sync_info = mybir.SyncInfo(
                    on_wait=rest[i:i + MAXW], on_update=[])
                orig_add(self, nop)
        orig_add(self, inst)

    tile_mod.TileContext._add_instruction = _add_instruction

    def _drain_and_barrier(self, tick_clock, wait_clock):
        drain_inst = self.nc.sync.drain()
        wait_clock.add_sem_waits(
            drain_inst.ins, ScopedClock({None: tick_clock.global_clock})
        )
        si = drain_inst.ins.sync_info
        waits = list(si.on_wait or []) if si is not None else []
        if len(waits) > MAXW:
            ups = list(si.on_update or []) if si is not None else []
            drain_inst.ins.sync_info = mybir.SyncInfo(on_wait=[], on_update=ups)
            for i in range(0, len(waits), MAXW):
                nop = self.nc.sync.nop(nofuse=True)
                nop.ins.sync_info = mybir.SyncInfo(
                    on_wait=waits[i:i + MAXW], on_update=[])
        self.nc.all_engine_barrier()
        assert self.sems is not None
        popped = self.nc._tile_sem_poison_stack.pop()
        assert popped is self._sem_poison
        self.nc.clear_and_free_semaphores(list(self.sems.allocated().values()))
        self.nc.all_engine_barrier()

    tile_mod.TileContext._drain_and_barrier = _drain_and_barrier
    tile_mod.TileContext._drain_split_patch = True


def _build_nc():
    import ml_dtypes
    import concourse.bass as bass
    import concourse.tile as tile
    from concourse import mybir
    from concourse.alu_op_type import AluOpType

    _patch_tile_drain()

    f32 = mybir.dt.float32
    bf16 = mybir.dt.bfloat16
    bf = ml_dtypes.bfloat16

    nc = bass.Bass("TRN2", target_bir_lowering=False, debug=False)

    xT = nc.dram_tensor("xT", [DIM, N], bf16, kind="ExternalInput").ap()
    ctxT = nc.dram_tensor("ctxT", [DIM, P], bf16, kind="ExternalInput").ap()
    biasF = nc.dram_tensor("biasF", [BIAS_TOTAL], bf16,
                           kind="ExternalInput").ap()
    wc = nc.dram_tensor("wc", [DIM, FL], bf16, kind="ExternalInput").ap()
    wkv = nc.dram_tensor("wkv", [DIM, 2 * DH], bf16, kind="ExternalInput").ap()
    wo = nc.dram_tensor("wo", [FL, DIM], bf16, kind="ExternalInput").ap()
    sdkc = nc.dram_tensor("sdkc", [128, 1], f32, kind="ExternalInput").ap()
    outT = nc.dram_tensor("outT", [DIM, N], bf16, kind="ExternalOutput").ap()

    mmain, mcorner, mthird = _masks()
    bandm_dram = nc.inline_tensor(mmain.astype(bf), "bandm").ap()
    corner_dram = nc.inline_tensor(mcorner.astype(bf), "cornm").ap()
    third_dram = nc.inline_tensor(mthird.astype(bf), "thirdm").ap()
    idup_np = (np.arange(128)[:, None] % 64 == np.arange(64)[None, :])
    idup_dram = nc.inline_tensor(idup_np.astype(bf), "idup").ap()
    # block-ones: col m sums partitions 64*(m//64) .. +64 -> row-broadcast ssq
    indb_np = (np.arange(128)[:, None] // 64
               == np.arange(128)[None, :] // 64)
    indb_dram = nc.inline_tensor(indb_np.astype(bf), "indb").ap()

    Exp = mybir.ActivationFunctionType.Exp
    Ln = mybir.ActivationFunctionType.Ln
    Sq = mybir.ActivationFunctionType.Square
    Cp = mybir.ActivationFunctionType.Copy

    with tile.TileContext(nc) as tc, \
            tc.tile_pool(name="big", bufs=1) as big, \
            tc.tile_pool(name="cst", bufs=1) as cst, \
            tc.tile_pool(name="sqp", bufs=2) as sqp, \
            tc.tile_pool(name="lnp", bufs=2) as lnp, \
            tc.tile_pool(name="bia", bufs=10) as biap, \
            tc.tile_pool(name="ptx", bufs=6) as ptxp, \
            tc.tile_pool(name="rcb", bufs=4) as rcbp, \
            tc.tile_pool(name="osb", bufs=3) as osbp, \
            tc.tile_pool(name="psS", bufs=2, space="PSUM") as psS, \
            tc.tile_pool(name="psA", bufs=2, space="PSUM") as psA:

        # ---- loads ----
        xr = xT.rearrange("(kt p) n -> p kt n", p=128)
        x_sb = big.tile([128, 8, N], bf16, tag="xT")
        for h in range(2):
            for k in range(4):
                nc.sync.dma_start(
                    x_sb[:, 2 * k:2 * k + 2, h * 512:(h + 1) * 512],
                    xr[:, 2 * k:2 * k + 2, h * 512:(h + 1) * 512])
        ctxr = ctxT.rearrange("(kt p) n -> p kt n", p=128)
        ctx_sb = big.tile([128, 8, P], bf16, tag="ctxT")
        for h in range(2):
            for k in range(4):
                nc.sync.dma_start(
                    ctx_sb[:, 2 * k:2 * k + 2, h * 512:(h + 1) * 512],
                    ctxr[:, 2 * k:2 * k + 2, h * 512:(h + 1) * 512])

        wcr = wc.rearrange("(kt p) f -> p kt f", p=128)
        wc_sb = big.tile([128, 8, FL], bf16, tag="wc")
        for ft in range(4):
            nc.scalar.dma_start(wc_sb[:, :, ft * 128:(ft + 1) * 128],
                                wcr[:, :, ft * 128:(ft + 1) * 128])
        wkv_sb = big.tile([128, 8, 2 * DH], bf16, tag="wkv")
        nc.scalar.dma_start(wkv_sb[:], wkv.rearrange("(kt p) f -> p kt f",
                                                     p=128))
        idup_sb = cst.tile([128, 64], bf16, tag="idup")
        nc.scalar.dma_start(idup_sb[:], idup_dram)
        indb_sb = cst.tile([128, 128], bf16, tag="indb")
        nc.scalar.dma_start(indb_sb[:], indb_dram)
        sdk_sb = cst.tile([128, 1], f32, tag="sdk")
        nc.scalar.dma_start(sdk_sb[:], sdkc)
        bandm_sb = cst.tile([128, 576], bf16, tag="bandm")
        nc.scalar.dma_start(bandm_sb[:], bandm_dram)
        corner_sb = cst.tile([128, 4 * WIND], bf16, tag="cornm")
        nc.scalar.dma_start(corner_sb[:], corner_dram)
        third_sb = cst.tile([128, 512], bf16, tag="thirdm")
        nc.scalar.dma_start(third_sb[:], third_dram)
        wo_sb = big.tile([128, 4, DIM], bf16, tag="wo")
        nc.scalar.dma_start(wo_sb[:], wo.rearrange("(ft p) e -> p ft e",
                                                   p=128))

        eps_sb = cst.tile([128, 1], f32, tag="eps")
        nc.vector.memset(eps_sb[:], 1e-24)
        garb_sb = cst.tile([128, 512], bf16, tag="garb")
        nc.vector.memset(garb_sb[:], 0.125)

        kvT_sb = big.tile([128, J], bf16, tag="kvT")   # k rows 0-63, v 64-127
        kdup_sb = big.tile([128, J], bf16, tag="kdup")  # k dup'd on 64-127
        qn_sb = big.tile([128, NQC, QW, 4], bf16, tag="qn")
        att_sb = big.tile([128, 4, N], bf16, tag="att")
        va_sb = big.tile([128, 16, 128], bf16, tag="va")  # [v | ones]
        nc.vector.memset(va_sb[:, :, DH:128], 1.0)
        rkT_sb = cst.tile([128, 16], f32, tag="rkT")
        ssqT_sb = cst.tile([128, 16], f32, tag="ssqT")

        # ---- PE warmup during loads ----
        for i in range(16):
            wps = psS.tile([128, 1024], f32, tag="S", name=f"warm{i}")
            nc.tensor.matmul(wps[:, 0:512], lhsT=garb_sb[:, 0:128],
                             rhs=garb_sb[:], start=True, stop=True)

        # ---- q projection + normalize (deferred one ft for overlap) ----
        def emit_q_mm(ft):
            ps = psS.tile([128, 1024], f32, tag="S", name=f"qps{ft}")
            for kt in range(8):
                for h in range(2):
                    nc.tensor.matmul(
                        ps[:, h * 512:(h + 1) * 512],
                        lhsT=wc_sb[:, kt, ft * 128:(ft + 1) * 128],
                        rhs=x_sb[:, kt, h * 512:(h + 1) * 512],
                        start=(kt == 0), stop=(kt == 7))
            return ps

        def emit_q_norm(ft, ps):
            sq = sqp.tile([128, 1024], bf16, tag="sq", name=f"qsq{ft}")
            nc.scalar.activation(sq[:], ps[:], Sq)
            ssq = psA.tile([128, 1024], f32, tag="avg", name=f"qssq{ft}")
            for h in range(2):
                nc.tensor.matmul(ssq[:, h * 512:(h + 1) * 512],
                                 lhsT=indb_sb[:],
                                 rhs=sq[:, h * 512:(h + 1) * 512],
                                 start=True, stop=True)
            lnq = lnp.tile([128, 1024], f32, tag="ln", name=f"qln{ft}")
            nc.scalar.activation(lnq[:], ssq[:], Ln, bias=eps_sb[:])
            rqb = lnp.tile([128, 1024], f32, tag="rqb", name=f"qrq{ft}")
            nc.scalar.activation(rqb[:], lnq[:], Exp, scale=-0.5)
            # qn = (q * sdk) * rsqrt(ssq), scattered to (qch, q, ft) layout
            nc.vector.scalar_tensor_tensor(
                out=qn_sb[:, :, :, ft],
                in0=ps[:].rearrange("p (c x) -> p c x", x=QW),
                scalar=sdk_sb[:],
                in1=rqb[:].rearrange("p (c x) -> p c x", x=QW),
                op0=AluOpType.mult, op1=AluOpType.mult)

        def emit_kv(jh):
            src = ctx_sb if jh == 0 else x_sb
            ps = psS.tile([128, 1024], f32, tag="S", name=f"kvps{jh}")
            for kt in range(8):
                for h in range(2):
                    nc.tensor.matmul(
                        ps[:, h * 512:(h + 1) * 512],
                        lhsT=wkv_sb[:, kt, :],
                        rhs=src[:, kt, h * 512:(h + 1) * 512],
                        start=(kt == 0), stop=(kt == 7))
            js = slice(jh * 1024, (jh + 1) * 1024)
            nc.vector.tensor_copy(out=kvT_sb[:, js], in_=ps[:])
            nc.scalar.activation(kdup_sb[64:128, js], ps[0:64, :], Cp)

        def emit_ktr(jh):
            """transpose k/v tiles of half jh; build va, rkT."""
            js0 = jh * 1024
            ktps = psA.tile([128, 1024], f32, tag="avg", name=f"ktp{jh}")
            ktv = ktps[:, 0:256].bitcast(bf16)       # [128, 512] bf16
            for i in range(8):
                nc.tensor.transpose(
                    ktv[:, i * 64:(i + 1) * 64],
                    kvT_sb[0:64, js0 + i * 128:js0 + (i + 1) * 128],
                    idup_sb[0:64, :])
            sqk = sqp.tile([128, 512], bf16, tag="sqk", name=f"sqk{jh}")
            nc.scalar.activation(sqk[:], ktv[:], Sq)
            nc.vector.tensor_reduce(
                ssqT_sb[:, jh * 8:(jh + 1) * 8],
                sqk[:].rearrange("p (t d) -> p t d", d=64),
                mybir.AxisListType.X, AluOpType.add)
            lnk = lnp.tile([128, 16], f32, tag="lnk", name=f"lnk{jh}")
            nc.scalar.activation(lnk[:, 0:8], ssqT_sb[:, jh * 8:(jh + 1) * 8],
                                 Ln, bias=eps_sb[:])
            nc.scalar.activation(rkT_sb[:, jh * 8:(jh + 1) * 8], lnk[:, 0:8],
                                 Exp, scale=-0.5)
            vtps = psA.tile([128, 1024], f32, tag="avg", name=f"vtp{jh}")
            vtv = vtps[:, 0:256].bitcast(bf16)
            for i in range(8):
                nc.tensor.transpose(
                    vtv[:, i * 64:(i + 1) * 64],
                    kvT_sb[64:128, js0 + i * 128:js0 + (i + 1) * 128],
                    idup_sb[64:128, :])
            nc.vector.tensor_copy(
                out=va_sb[:, jh * 8:(jh + 1) * 8, 0:DH],
                in_=vtv[:].rearrange("p (t d) -> p t d", d=64))

        qpss = {}
        qpss[0] = emit_q_mm(0)
        qpss[1] = emit_q_mm(1)
        emit_q_norm(0, qpss[0])
        emit_kv(0)
        emit_q_norm(1, qpss[1])
        qpss[2] = emit_q_mm(2)
        emit_ktr(0)
        qpss[3] = emit_q_mm(3)
        emit_q_norm(2, qpss[2])
        emit_kv(1)
        emit_q_norm(3, qpss[3])
        emit_ktr(1)

        # ---- fused attention per query chunk ----
        def emit_sim(qch, g2, jt, qlo, qhi, kind, ct=None):
            """sim matmuls + exp(scale=1/||k||) + bias/mask mult -> ptx."""
            w = qhi - qlo
            base = 64 * g2
            lhs = kvT_sb if g2 == 0 else kdup_sb
            q0 = qch * QW
            w4 = 4 * w
            ps = psS.tile([128, 1024], f32, tag="S",
                          name=f"sps{qch}{g2}{jt}{kind}")
            qv = qn_sb[base:base + 64, qch, :, :].rearrange(
                "p x f -> p (x f)")
            for lo in range(0, w4, 512):
                hi = min(w4, lo + 512)
                s0 = 4 * (qlo - q0)
                nc.tensor.matmul(
                    ps[:, lo:hi],
                    lhsT=lhs[base:base + 64, jt * 128:(jt + 1) * 128],
                    rhs=qv[:, s0 + lo:s0 + hi],
                    start=True, stop=True)
            pt = ptxp.tile([128, 1024], bf16, tag="ptx",
                           name=f"ptx{qch}{g2}{jt}{kind}")
            nc.scalar.activation(pt[:, 0:w4], ps[:, 0:w4], Exp,
                                 scale=rkT_sb[:, jt:jt + 1])
            if kind == "x":
                off, bw = BIAS_OFFS[(qch, g2, ct)]
                assert bw == w
                bt = biap.tile([128, 1024], bf16, tag="bias",
                               name=f"bt{qch}{g2}{jt}")
                nc.sync.dma_start(
                    bt[:, 0:w4],
                    biasF[off:off + 128 * w4].rearrange("(p x) -> p x", p=128))
                nc.vector.tensor_mul(pt[:, 0:w4], pt[:, 0:w4], bt[:, 0:w4])
            else:
                msk = {"corner": corner_sb, "main": bandm_sb,
                       "third": third_sb}[kind]
                nc.vector.tensor_mul(pt[:, 0:w4], pt[:, 0:w4], msk[:, 0:w4])
            return pt

        def emit_av(avps, qch, g2, jt, qlo, qhi, pt, start, stop):
            q0 = qch * QW
            w4 = 4 * (qhi - qlo)
            c0 = 4 * (qlo - q0)
            cuts = sorted({c0, c0 + w4}
                          | {b for b in (512,) if c0 < b < c0 + w4})
            for lo, hi in zip(cuts[:-1], cuts[1:]):
                nc.tensor.matmul(avps[:, lo:hi],
                                 lhsT=va_sb[:, jt, :],
                                 rhs=pt[:, lo - c0:hi - c0],
                                 start=start, stop=stop)

        def emit_attnorm(avps, qch, g2):
            q0 = qch * QW
            lnd = rcbp.tile([64, 1024], f32, tag="lnd",
                            name=f"lnd{qch}{g2}")
            nc.scalar.activation(lnd[:], avps[64:128, :], Ln)
            rb = rcbp.tile([64, 1024], f32, tag="rcb",
                           name=f"rcb{qch}{g2}")
            nc.scalar.activation(rb[:], lnd[:], Exp, scale=-1.0)
            nc.vector.tensor_mul(
                att_sb[64 * g2:64 * g2 + 64, :, q0:q0 + QW],
                avps[0:64, :].rearrange("p (x f) -> p f x", f=4),
                rb[:].rearrange("p (x f) -> p f x", f=4))

        def emit_outproj(qc):
            for et in range(8):
                ops = psS.tile([128, 1024], f32, tag="S", name=f"op{qc}{et}")
                for ftile in range(4):
                    nc.tensor.matmul(
                        ops[:, 0:512],
                        lhsT=wo_sb[:, ftile, et * 128:(et + 1) * 128],
                        rhs=att_sb[:, ftile, qc * 512:(qc + 1) * 512],
                        start=(ftile == 0), stop=(ftile == 3))
                o = osbp.tile([128, 512], bf16, tag="osb", name=f"o{qc}{et}")
                if et % 2 == 0:
                    nc.vector.tensor_copy(out=o[:], in_=ops[:, 0:512])
                else:
                    nc.scalar.activation(o[:], ops[:, 0:512], Cp)
                nc.gpsimd.dma_start(
                    out=outT[et * 128:(et + 1) * 128,
                             qc * 512:(qc + 1) * 512],
                    in_=o[:])

        deferred = {0: [], 1: []}
        for qch in range(NQC):
            for g2 in range(2):
                avps = None
                xs = _x_units(qch)
                units = ([("x",) + xs[0]]
                         + [("b", jt, lo, hi, kk)
                            for jt, lo, hi, kk in _band_units(qch)]
                         + [("x",) + u for u in xs[1:]])
                pend = None
                for i, u in enumerate(units):
                    if u[0] == "x":
                        _, ct, qlo, qhi = u
                        jt, kind = 8 + ct, "x"
                        pt = emit_sim(qch, g2, jt, qlo, qhi, kind, ct=ct)
                    else:
                        _, jt, qlo, qhi, kind = u
                        pt = emit_sim(qch, g2, jt, qlo, qhi, kind)
                    # run the previous chunk's normalize/out-proj behind this
                    # group's first sim so the reciprocal chain and the pool
                    # WAR on the av tile are off the PE critical path
                    if i == 0 and deferred[g2]:
                        for fn in deferred[g2]:
                            fn()
                        deferred[g2] = []
                    if pend is not None:
                        if avps is None:
                            avps = psA.tile([128, 1024], f32, tag="avg",
                                            name=f"av{qch}{g2}")
                        emit_av(avps, qch, g2, *pend, start=(i == 1),
                                stop=False)
                    pend = (jt, qlo, qhi, pt)
                emit_av(avps, qch, g2, *pend, start=False, stop=True)
                deferred[g2] = [
                    lambda a=avps, q=qch, g=g2: emit_attnorm(a, q, g)]
                if qch == 1 and g2 == 1:
                    deferred[g2].append(lambda: emit_outproj(0))
        for g2 in range(2):
            for fn in deferred[g2]:
                fn()
        emit_outproj(1)

    return nc


_NC = None


def _get_nc():
    global _NC
    if _NC is None:
        _NC = _build_nc()
    return _NC


def _prep_in_maps(x, prefix_context, attn_bias, gamma, Wq, Wkv, q_scale,
                  k_scale, Wo, mask):
    import ml_dtypes
    bf = ml_dtypes.bfloat16

    x = np.asarray(x, np.float32)
    prefix_context = np.asarray(prefix_context, np.float32)
    attn_bias = np.asarray(attn_bias, np.float32)
    gamma = np.asarray(gamma, np.float32)
    Wq = np.asarray(Wq, np.float32)
    Wkv = np.asarray(Wkv, np.float32)
    q_scale = np.asarray(q_scale, np.float32)
    k_scale = np.asarray(k_scale, np.float32)
    Wo = np.asarray(Wo, np.float32)
    mask = np.asarray(mask)

    tril = np.triu(np.ones((N, N), np.float32))  # keep key c <= query i [c, i]
    sdk = (8.0 * q_scale * k_scale).astype(np.float32)
    sdkc = np.tile(sdk, 2).reshape(128, 1)
    wkvT = np.ascontiguousarray(Wkv.T).astype(bf)

    in_maps = []
    for c in CORES:
        b, g = c // 2, c % 2
        hs = slice(g * HL, (g + 1) * HL)
        # exp(bias) with causal kill and key-mask folded in, [h, c, i] layout
        eb = np.exp(attn_bias[hs].transpose(0, 2, 1)) * tril[None]
        maskf = mask[b].astype(np.float32)
        if not maskf.all():
            eb = eb * maskf[None, :, None]
        eb = eb.astype(bf)
        # pack bias tiles in kernel consumption order
        bflat = np.empty(BIAS_TOTAL, bf)
        for qch in range(NQC):
            for g2 in range(2):
                heads = [2 * ft + g2 for ft in range(4)]
                for ct, qlo, qhi in _x_units(qch):
                    off, w = BIAS_OFFS[(qch, g2, ct)]
                    t = eb[heads, 128 * ct:128 * (ct + 1), qlo:qhi]
                    bflat[off:off + 128 * 4 * w] = (
                        t.transpose(1, 2, 0).ravel())
        Wg = Wq[g * FL:(g + 1) * FL] * gamma[None, :]
        s = Wg.sum(axis=1)
        wcT = Wg.T - s[None, :] / DIM
        wog = Wo[:, g * FL:(g + 1) * FL]
        in_maps.append(dict(
            xT=np.ascontiguousarray(x[b].T).astype(bf),
            ctxT=np.ascontiguousarray(prefix_context[b].T).astype(bf),
            biasF=bflat,
            wc=np.ascontiguousarray(wcT).astype(bf),
            wkv=wkvT,
            wo=np.ascontiguousarray(wog.T).astype(bf),
            sdkc=sdkc,
        ))
    return in_maps


def kernel(**inputs):
    from concourse.bass_utils import run_bass_kernel_spmd

    nc = _get_nc()
    in_maps = _prep_in_maps(**inputs)
    res = run_bass_kernel_spmd(nc, in_maps, CORES).results
    out = np.empty((B, N, DIM), np.float32)
    for b in range(B):
        out[b] = (np.asarray(res[2 * b]["outT"]).astype(np.float32)
                  + np.asarray(res[2 * b + 1]["outT"]).astype(np.float32)).T
    return out


# revision 20
# speedup vs baseline: 1.2805x; 1.0476x over previous
# Distributed sparse-attention kernel for Trainium2 (8 NeuronCores).
#
# Sharding: core c = (batch b = c//2, head-group g = c%2 of 8 heads).
# Per core, heads are split into two PE partition groups g2 = h%2 (even heads
# on partitions 0-63, odd on 64-127) with ft = h//2 indexing the 4 heads of a
# group.  Attention is computed per 256-query chunk, fused sim->exp->mask->AV:
#   q   = meancenter(x) @ Wc            (LN folded into Wc on host)
#   kv  = [prefix; x] @ Wkv.T           (MQA single head)
#   qn  = q * (8*q_scale*k_scale) * rsqrt(sumsq(q))   (per-query bcast via a
#         block-ones reduce matmul so no partition-broadcast is needed)
#   kn  = raw k; 1/||k|| folded into the Exp activation's per-partition scale
#   P   = exp(kn.T qn * rk) * exp_bias  (bias/causal folded into a
#         multiplicative exp(bias) table, host-packed per 128-key tile)
#   AV  uses va = [v | ones] as stationary so PSUM rows 64-127 hold the
#         softmax denominator pre-broadcast; normalize = reciprocal + mult
#   out = Wo_g.T @ att, summed across the two head-group cores on host.

import numpy as np

B, N, P, DIM, HEADS, DH = 4, 1024, 1024, 1024, 16, 64
HL = 8                 # heads per core
FL = HL * DH           # 512 local q features
J = P + N              # 2048 keys
WIND = 16              # prefix cond-window
QW = 256               # query chunk
NQC = N // QW          # 4 query chunks
CORES = list(range(8))


def _x_units(qch):
    """x-region key tiles for query chunk qch: (ct, qlo, qhi)."""
    q0 = qch * QW
    return [(ct, max(q0, 128 * ct), q0 + QW) for ct in range(2 * qch + 2)]


def _band_units(qch):
    """prefix band tiles: (jt, qlo, qhi, kind).  The "third" tile extends
    WIND-1 queries into the next chunk (the corner of tile 2qch+1), whose AV
    contribution is carried into the next chunk's accumulator."""
    q0 = qch * QW
    return [(2 * qch, q0, q0 + 144, "main"),
            (2 * qch + 1, q0 + 128, min(q0 + 256 + WIND, N), "third")]


def _bias_layout():
    """Offsets of host-packed bias tiles keyed (qch, g2, ct); emission order
    must match _prep_in_maps' packing order exactly."""
    offs = {}
    off = 0
    for qch in range(NQC):
        for g2 in range(2):
            for ct, qlo, qhi in _x_units(qch):
                w = qhi - qlo
                offs[(qch, g2, ct)] = (off, w)
                off += 128 * 4 * w
    return offs, off


BIAS_OFFS, BIAS_TOTAL = _bias_layout()


def _masks():
    """(q, ft)-major band mask: one 144-wide pattern serves every band tile
    (key r of any prefix tile attends relative queries r..r+WIND-1)."""
    r = np.arange(128)[:, None]
    t = np.arange(144)[None, :]
    main = ((t - r >= 0) & (t - r < WIND)).astype(np.float32)
    return np.repeat(main[:, :, None], 4, axis=2).reshape(128, 576)


def _patch_tile_drain():
    """walrus in this image only encodes ~2 sem waits on a CTRL (Drain/Nop)
    instruction; Tile's exit drain attaches every outstanding sem wait to a
    single drain.  Split the waits across extra sync-engine nops."""
    import concourse.tile as tile_mod
    from concourse import mybir
    from concourse.vector_clock import ScopedClock

    if getattr(tile_mod.TileContext, "_drain_split_patch", False):
        return
    MAXW = 1

    _ENGS = {
        mybir.EngineType.PE, mybir.EngineType.Activation,
        mybir.EngineType.Pool, mybir.EngineType.DVE, mybir.EngineType.SP,
    }
    _LIMITS = {}
    _nsplit = [0]
    orig_add = tile_mod.TileContext._add_instruction

    def _add_instruction(self, inst):
        si = inst.sync_info
        lim = _LIMITS.get(inst.engine, 1)
        if (si is not None and si.on_wait and len(si.on_wait) > lim
                and inst.engine in _ENGS):
            waits = list(si.on_wait)
            keep = waits[:lim]
            rest = waits[lim:]
            inst.sync_info = mybir.SyncInfo(
                on_wait=keep, on_update=list(si.on_update or []))
            for i in range(0, len(rest), MAXW):
                _nsplit[0] += 1
                nop = mybir.InstNoOp(
                    name=f"{inst.name}-ws{_nsplit[0]}", ins=[], outs=[])
                nop.engine = inst.engine
                nop.sync_info = mybir.SyncInfo(
                    on_wait=rest[i:i + MAXW], on_update=[])
                orig_add(self, nop)
        orig_add(self, inst)

    tile_mod.TileContext._add_instruction = _add_instruction

    def _drain_and_barrier(self, tick_clock, wait_clock):
        drain_inst = self.nc.sync.drain()
        wait_clock.add_sem_waits(
            drain_inst.ins, ScopedClock({None: tick_clock.global_clock})
        )
        si = drain_inst.ins.sync_info
        waits = list(si.on_wait or []) if si is not None else []
        if len(waits) > MAXW:
            ups = list(si.on_update or []) if si is not None else []
            drain_inst.ins.sync_info = mybir.SyncInfo(on_wait=[], on_update=ups)
            for i in range(0, len(waits), MAXW):
                nop = self.nc.sync.nop(nofuse=True)
                nop.ins.sync_info = mybir.SyncInfo(
                    on_wait=waits[i:i + MAXW], on_update=[])
        self.nc.all_engine_barrier()
        assert self.sems is not None
        popped = self.nc._tile_sem_poison_stack.pop()
        assert popped is self._sem_poison
        self.nc.clear_and_free_semaphores(list(self.sems.allocated().values()))
        self.nc.all_engine_barrier()

    tile_mod.TileContext._drain_and_barrier = _drain_and_barrier
    tile_mod.TileContext._drain_split_patch = True


def _build_nc():
    import ml_dtypes
    import concourse.bass as bass
    import concourse.tile as tile
    from concourse import mybir
    from concourse.alu_op_type import AluOpType

    _patch_tile_drain()

    f32 = mybir.dt.float32
    bf16 = mybir.dt.bfloat16
    bf = ml_dtypes.bfloat16

    nc = bass.Bass("TRN2", target_bir_lowering=False, debug=False)

    xT = nc.dram_tensor("xT", [DIM, N], bf16, kind="ExternalInput").ap()
    ctxT = nc.dram_tensor("ctxT", [DIM, P], bf16, kind="ExternalInput").ap()
    biasF = nc.dram_tensor("biasF", [BIAS_TOTAL], bf16,
                           kind="ExternalInput").ap()
    wc = nc.dram_tensor("wc", [DIM, FL], bf16, kind="ExternalInput").ap()
    wkv = nc.dram_tensor("wkv", [DIM, 2 * DH], bf16, kind="ExternalInput").ap()
    wo = nc.dram_tensor("wo", [FL, DIM], bf16, kind="ExternalInput").ap()
    sdkc = nc.dram_tensor("sdkc", [128, 1], f32, kind="ExternalInput").ap()
    outT = nc.dram_tensor("outT", [DIM, N], bf16, kind="ExternalOutput").ap()

    bandm_dram = nc.inline_tensor(_masks().astype(bf), "bandm").ap()
    idup_np = (np.arange(128)[:, None] % 64 == np.arange(64)[None, :])
    idup_dram = nc.inline_tensor(idup_np.astype(bf), "idup").ap()
    # block-ones: col m sums partitions 64*(m//64) .. +64 -> row-broadcast ssq
    indb_np = (np.arange(128)[:, None] // 64
               == np.arange(128)[None, :] // 64)
    indb_dram = nc.inline_tensor(indb_np.astype(bf), "indb").ap()

    Exp = mybir.ActivationFunctionType.Exp
    Ln = mybir.ActivationFunctionType.Ln
    Sq = mybir.ActivationFunctionType.Square
    Cp = mybir.ActivationFunctionType.Copy

    with tile.TileContext(nc) as tc, \
            tc.tile_pool(name="big", bufs=1) as big, \
            tc.tile_pool(name="cst", bufs=1) as cst, \
            tc.tile_pool(name="sqp", bufs=2) as sqp, \
            tc.tile_pool(name="lnp", bufs=2) as lnp, \
            tc.tile_pool(name="bia", bufs=10) as biap, \
            tc.tile_pool(name="ptx", bufs=6) as ptxp, \
            tc.tile_pool(name="ptc", bufs=2) as ptcp, \
            tc.tile_pool(name="rcb", bufs=4) as rcbp, \
            tc.tile_pool(name="osb", bufs=3) as osbp, \
            tc.tile_pool(name="psS", bufs=2, space="PSUM") as psS, \
            tc.tile_pool(name="psA", bufs=2, space="PSUM") as psA:

        # ---- loads ----
        xr = xT.rearrange("(kt p) n -> p kt n", p=128)
        x_sb = big.tile([128, 8, N], bf16, tag="xT")
        for h in range(2):
            nc.sync.dma_start(x_sb[:, :, h * 512:(h + 1) * 512],
                              xr[:, :, h * 512:(h + 1) * 512])
        ctxr = ctxT.rearrange("(kt p) n -> p kt n", p=128)
        ctx_sb = big.tile([128, 8, P], bf16, tag="ctxT")
        for h in range(2):
            nc.gpsimd.dma_start(ctx_sb[:, :, h * 512:(h + 1) * 512],
                                ctxr[:, :, h * 512:(h + 1) * 512])

        wcr = wc.rearrange("(kt p) f -> p kt f", p=128)
        wc_sb = big.tile([128, 8, FL], bf16, tag="wc")
        for ft in range(4):
            nc.scalar.dma_start(wc_sb[:, :, ft * 128:(ft + 1) * 128],
                                wcr[:, :, ft * 128:(ft + 1) * 128])
        wkv_sb = big.tile([128, 8, 2 * DH], bf16, tag="wkv")
        nc.scalar.dma_start(wkv_sb[:], wkv.rearrange("(kt p) f -> p kt f",
                                                     p=128))
        idup_sb = cst.tile([128, 64], bf16, tag="idup")
        nc.scalar.dma_start(idup_sb[:], idup_dram)
        indb_sb = cst.tile([128, 128], bf16, tag="indb")
        nc.scalar.dma_start(indb_sb[:], indb_dram)
        sdk_sb = cst.tile([128, 1], f32, tag="sdk")
        nc.scalar.dma_start(sdk_sb[:], sdkc)
        bandm_sb = cst.tile([128, 576], bf16, tag="bandm")
        nc.scalar.dma_start(bandm_sb[:], bandm_dram)
        wo_sb = big.tile([128, 4, DIM], bf16, tag="wo")
        nc.scalar.dma_start(wo_sb[:], wo.rearrange("(ft p) e -> p ft e",
                                                   p=128))

        eps_sb = cst.tile([128, 1], f32, tag="eps")
        nc.vector.memset(eps_sb[:], 1e-24)
        garb_sb = cst.tile([128, 512], bf16, tag="garb")
        nc.vector.memset(garb_sb[:], 0.125)

        kvT_sb = big.tile([128, J], bf16, tag="kvT")   # k rows 0-63, v 64-127
        kdup_sb = big.tile([128, J], bf16, tag="kdup")  # k dup'd on 64-127
        qn_sb = big.tile([128, NQC, QW, 4], bf16, tag="qn")
        att_sb = big.tile([128, 4, N], bf16, tag="att")
        va_sb = big.tile([128, 16, 128], bf16, tag="va")  # [v | ones]
        nc.vector.memset(va_sb[:, :, DH:128], 1.0)
        rkT_sb = cst.tile([128, 16], f32, tag="rkT")
        ssqT_sb = cst.tile([128, 16], f32, tag="ssqT")

        # ---- PE warmup during loads ----
        for i in range(16):
            wps = psS.tile([128, 1024], f32, tag="S", name=f"warm{i}")
            nc.tensor.matmul(wps[:, 0:512], lhsT=garb_sb[:, 0:128],
                             rhs=garb_sb[:], start=True, stop=True)

        # ---- q projection + normalize (deferred one ft for overlap) ----
        def emit_q_mm(ft):
            ps = psS.tile([128, 1024], f32, tag="S", name=f"qps{ft}")
            for kt in range(8):
                for h in range(2):
                    nc.tensor.matmul(
                        ps[:, h * 512:(h + 1) * 512],
                        lhsT=wc_sb[:, kt, ft * 128:(ft + 1) * 128],
                        rhs=x_sb[:, kt, h * 512:(h + 1) * 512],
                        start=(kt == 0), stop=(kt == 7))
            return ps

        def emit_q_norm(ft, ps):
            qf = lnp.tile([128, 1024], bf16, tag="qf", name=f"qf{ft}")
            nc.vector.tensor_copy(out=qf[:], in_=ps[:])
            sq = sqp.tile([128, 1024], bf16, tag="sq", name=f"qsq{ft}")
            nc.scalar.activation(sq[:], qf[:], Sq)
            ssq = psA.tile([128, 1024], f32, tag="avg", name=f"qssq{ft}")
            for h in range(2):
                nc.tensor.matmul(ssq[:, h * 512:(h + 1) * 512],
                                 lhsT=indb_sb[:],
                                 rhs=sq[:, h * 512:(h + 1) * 512],
                                 start=True, stop=True)
            lnq = lnp.tile([128, 1024], f32, tag="ln", name=f"qln{ft}")
            nc.scalar.activation(lnq[:], ssq[:], Ln, bias=eps_sb[:])
            rqb = lnp.tile([128, 1024], f32, tag="rqb", name=f"qrq{ft}")
            nc.scalar.activation(rqb[:], lnq[:], Exp, scale=-0.5)
            # qn = (q * sdk) * rsqrt(ssq), scattered to (qch, q, ft) layout
            nc.vector.scalar_tensor_tensor(
                out=qn_sb[:, :, :, ft],
                in0=qf[:].rearrange("p (c x) -> p c x", x=QW),
                scalar=sdk_sb[:],
                in1=rqb[:].rearrange("p (c x) -> p c x", x=QW),
                op0=AluOpType.mult, op1=AluOpType.mult)

        def emit_kv(jh):
            src = ctx_sb if jh == 0 else x_sb
            ps = psS.tile([128, 1024], f32, tag="S", name=f"kvps{jh}")
            for kt in range(8):
                for h in range(2):
                    nc.tensor.matmul(
                        ps[:, h * 512:(h + 1) * 512],
                        lhsT=wkv_sb[:, kt, :],
                        rhs=src[:, kt, h * 512:(h + 1) * 512],
                        start=(kt == 0), stop=(kt == 7))
            js = slice(jh * 1024, (jh + 1) * 1024)
            nc.vector.tensor_copy(out=kvT_sb[:, js], in_=ps[:])
            nc.vector.tensor_copy(out=kdup_sb[64:128, js], in_=ps[0:64, :])

        def emit_ktr(jh):
            """transpose k/v tiles of half jh; build va, rkT."""
            js0 = jh * 1024
            ktps = psA.tile([128, 1024], f32, tag="avg", name=f"ktp{jh}")
            ktv = ktps[:, 0:256].bitcast(bf16)       # [128, 512] bf16
            for i in range(8):
                nc.tensor.transpose(
                    ktv[:, i * 64:(i + 1) * 64],
                    kvT_sb[0:64, js0 + i * 128:js0 + (i + 1) * 128],
                    idup_sb[0:64, :])
            sqk = sqp.tile([128, 512], bf16, tag="sqk", name=f"sqk{jh}")
            nc.scalar.activation(sqk[:], ktv[:], Sq)
            nc.vector.tensor_reduce(
                ssqT_sb[:, jh * 8:(jh + 1) * 8],
                sqk[:].rearrange("p (t d) -> p t d", d=64),
                mybir.AxisListType.X, AluOpType.add)
            lnk = lnp.tile([128, 16], f32, tag="lnk", name=f"lnk{jh}")
            nc.scalar.activation(lnk[:, 0:8], ssqT_sb[:, jh * 8:(jh + 1) * 8],
                                 Ln, bias=eps_sb[:])
            nc.scalar.activation(rkT_sb[:, jh * 8:(jh + 1) * 8], lnk[:, 0:8],
                                 Exp, scale=-0.5)
            vtps = psA.tile([128, 1024], f32, tag="avg", name=f"vtp{jh}")
            vtv = vtps[:, 0:256].bitcast(bf16)
            for i in range(8):
                nc.tensor.transpose(
                    vtv[:, i * 64:(i + 1) * 64],
                    kvT_sb[64:128, js0 + i * 128:js0 + (i + 1) * 128],
                    idup_sb[64:128, :])
            nc.vector.tensor_copy(
                out=va_sb[:, jh * 8:(jh + 1) * 8, 0:DH],
                in_=vtv[:].rearrange("p (t d) -> p t d", d=64))

        qpss = {}
        qpss[0] = emit_q_mm(0)
        qpss[1] = emit_q_mm(1)
        emit_q_norm(0, qpss[0])
        emit_kv(0)
        emit_q_norm(1, qpss[1])
        qpss[2] = emit_q_mm(2)
        emit_ktr(0)
        qpss[3] = emit_q_mm(3)
        emit_q_norm(2, qpss[2])
        emit_kv(1)
        emit_q_norm(3, qpss[3])
        emit_ktr(1)

        # ---- fused attention per query chunk ----
        def emit_sim(qch, g2, jt, qlo, qhi, kind, ct=None):
            """sim matmuls + exp(scale=1/||k||) + bias/mask mult -> ptx."""
            w = qhi - qlo
            base = 64 * g2
            lhs = kvT_sb if g2 == 0 else kdup_sb
            q0 = qch * QW
            w4 = 4 * w
            ps = psS.tile([128, 1024], f32, tag="S",
                          name=f"sps{qch}{g2}{jt}{kind}")
            qv = qn_sb[base:base + 64, :, :, :].rearrange(
                "p c x f -> p (c x f)")
            for lo in range(0, w4, 512):
                hi = min(w4, lo + 512)
                nc.tensor.matmul(
                    ps[:, lo:hi],
                    lhsT=lhs[base:base + 64, jt * 128:(jt + 1) * 128],
                    rhs=qv[:, 4 * qlo + lo:4 * qlo + hi],
                    start=True, stop=True)
            pool, tag = ((ptxp, "ptx") if kind != "third"
                         else (ptcp, "ptc"))
            pt = pool.tile([128, 1024], bf16, tag=tag,
                           name=f"ptx{qch}{g2}{jt}{kind}")
            nc.scalar.activation(pt[:, 0:w4], ps[:, 0:w4], Exp,
                                 scale=rkT_sb[:, jt:jt + 1])
            if kind == "x":
                off, bw = BIAS_OFFS[(qch, g2, ct)]
                assert bw == w
                bt = biap.tile([128, 1024], bf16, tag="bias",
                               name=f"bt{qch}{g2}{jt}")
                nc.sync.dma_start(
                    bt[:, 0:w4],
                    biasF[off:off + 128 * w4].rearrange("(p x) -> p x", p=128))
                nc.vector.tensor_mul(pt[:, 0:w4], pt[:, 0:w4], bt[:, 0:w4])
            else:
                nc.vector.tensor_mul(pt[:, 0:w4], pt[:, 0:w4],
                                     bandm_sb[:, 0:w4])
            return pt

        def emit_av(avps, q0, jt, qlo, qhi, pt, start, stop, pt0=None):
            c0 = 4 * (qlo - q0)
            pt0 = 0 if pt0 is None else 4 * pt0
            cuts = sorted({c0, 4 * (qhi - q0)}
                          | {b for b in (512,) if c0 < b < 4 * (qhi - q0)})
            for lo, hi in zip(cuts[:-1], cuts[1:]):
                nc.tensor.matmul(avps[:, lo:hi],
                                 lhsT=va_sb[:, jt, :],
                                 rhs=pt[:, pt0 + lo - c0:pt0 + hi - c0],
                                 start=start, stop=stop)

        def emit_attnorm(avps, qch, g2):
            q0 = qch * QW
            lnd = rcbp.tile([64, 1024], f32, tag="lnd",
                            name=f"lnd{qch}{g2}")
            nc.scalar.activation(lnd[:], avps[64:128, :], Ln)
            rb = rcbp.tile([64, 1024], f32, tag="rcb",
                           name=f"rcb{qch}{g2}")
            nc.scalar.activation(rb[:], lnd[:], Exp, scale=-1.0)
            nc.vector.tensor_mul(
                att_sb[64 * g2:64 * g2 + 64, :, q0:q0 + QW],
                avps[0:64, :].rearrange("p (x f) -> p f x", f=4),
                rb[:].rearrange("p (x f) -> p f x", f=4))

        def emit_outproj(qc, ets=range(8)):
            for et in ets:
                ops = psS.tile([128, 1024], f32, tag="S", name=f"op{qc}{et}")
                for ftile in range(4):
                    nc.tensor.matmul(
                        ops[:, 0:512],
                        lhsT=wo_sb[:, ftile, et * 128:(et + 1) * 128],
                        rhs=att_sb[:, ftile, qc * 512:(qc + 1) * 512],
                        start=(ftile == 0), stop=(ftile == 3))
                o = osbp.tile([128, 512], bf16, tag="osb", name=f"o{qc}{et}")
                nc.vector.tensor_copy(out=o[:], in_=ops[:, 0:512])
                nc.gpsimd.dma_start(
                    out=outT[et * 128:(et + 1) * 128,
                             qc * 512:(qc + 1) * 512],
                    in_=o[:])

        deferred = {0: [], 1: []}
        carry = {0: None, 1: None}
        for qch in range(NQC):
            q0 = qch * QW
            for g2 in range(2):
                avps = None
                xs = _x_units(qch)
                units = ([("x",) + xs[0]]
                         + [("b", jt, lo, hi, kk)
                            for jt, lo, hi, kk in _band_units(qch)]
                         + [("x",) + u for u in xs[1:]])
                pend = None
                for i, u in enumerate(units):
                    if u[0] == "x":
                        _, ct, qlo, qhi = u
                        jt, kind = 8 + ct, "x"
                        pt = emit_sim(qch, g2, jt, qlo, qhi, kind, ct=ct)
                    else:
                        _, jt, qlo, qhi, kind = u
                        pt = emit_sim(qch, g2, jt, qlo, qhi, kind)
                    # run the previous chunk's normalize/out-proj behind this
                    # group's first sim so the normalize chain and the pool
                    # WAR on the av tile are off the PE critical path
                    if i == 0 and deferred[g2]:
                        for fn in deferred[g2]:
                            fn()
                        deferred[g2] = []
                    if qch == 3 and g2 == 1 and i == 0:
                        for fn in deferred[0]:
                            fn()
                        deferred[0] = []
                    if pend is not None:
                        if avps is None:
                            avps = psA.tile([128, 1024], f32, tag="avg",
                                            name=f"av{qch}{g2}")
                        emit_av(avps, q0, *pend, start=(i == 1), stop=False)
                        if i == 1 and carry[g2] is not None:
                            cjt, cpt = carry[g2]
                            emit_av(avps, q0, cjt, q0, q0 + WIND, cpt,
                                    start=False, stop=False, pt0=128)
                            carry[g2] = None
                    if u[0] == "b" and kind == "third" and qhi > q0 + QW:
                        carry[g2] = (jt, pt)
                        pend = (jt, qlo, q0 + QW, pt)
                    else:
                        pend = (jt, qlo, qhi, pt)
                emit_av(avps, q0, *pend, start=False, stop=True)
                deferred[g2] = [
                    lambda a=avps, q=qch, g=g2: emit_attnorm(a, q, g)]
                if qch == 1 and g2 == 1:
                    deferred[g2].append(lambda: emit_outproj(0, range(0, 4)))
                if qch == 2 and g2 == 0:
                    deferred[g2].append(lambda: emit_outproj(0, range(4, 8)))
        for g2 in range(2):
            for fn in deferred[g2]:
                fn()
        emit_outproj(1)

    return nc


_NC = None


def _get_nc():
    global _NC
    if _NC is None:
        _NC = _build_nc()
    return _NC


def _prep_in_maps(x, prefix_context, attn_bias, gamma, Wq, Wkv, q_scale,
                  k_scale, Wo, mask):
    import ml_dtypes
    bf = ml_dtypes.bfloat16

    x = np.asarray(x, np.float32)
    prefix_context = np.asarray(prefix_context, np.float32)
    attn_bias = np.asarray(attn_bias, np.float32)
    gamma = np.asarray(gamma, np.float32)
    Wq = np.asarray(Wq, np.float32)
    Wkv = np.asarray(Wkv, np.float32)
    q_scale = np.asarray(q_scale, np.float32)
    k_scale = np.asarray(k_scale, np.float32)
    Wo = np.asarray(Wo, np.float32)
    mask = np.asarray(mask)

    tril = np.triu(np.ones((N, N), np.float32))  # keep key c <= query i [c, i]
    sdk = (8.0 * q_scale * k_scale).astype(np.float32)
    sdkc = np.tile(sdk, 2).reshape(128, 1)
    wkvT = np.ascontiguousarray(Wkv.T).astype(bf)

    in_maps = []
    for c in CORES:
        b, g = c // 2, c % 2
        hs = slice(g * HL, (g + 1) * HL)
        # exp(bias) with causal kill and key-mask folded in, [h, c, i] layout
        eb = np.exp(attn_bias[hs].transpose(0, 2, 1)) * tril[None]
        maskf = mask[b].astype(np.float32)
        if not maskf.all():
            eb = eb * maskf[None, :, None]
        eb = eb.astype(bf)
        # pack bias tiles in kernel consumption order
        bflat = np.empty(BIAS_TOTAL, bf)
        for qch in range(NQC):
            for g2 in range(2):
                heads = [2 * ft + g2 for ft in range(4)]
                for ct, qlo, qhi in _x_units(qch):
                    off, w = BIAS_OFFS[(qch, g2, ct)]
                    t = eb[heads, 128 * ct:128 * (ct + 1), qlo:qhi]
                    bflat[off:off + 128 * 4 * w] = (
                        t.transpose(1, 2, 0).ravel())
        Wg = Wq[g * FL:(g + 1) * FL] * gamma[None, :]
        s = Wg.sum(axis=1)
        wcT = Wg.T - s[None, :] / DIM
        wog = Wo[:, g * FL:(g + 1) * FL]
        in_maps.append(dict(
            xT=np.ascontiguousarray(x[b].T).astype(bf),
            ctxT=np.ascontiguousarray(prefix_context[b].T).astype(bf),
            biasF=bflat,
            wc=np.ascontiguousarray(wcT).astype(bf),
            wkv=wkvT,
            wo=np.ascontiguousarray(wog.T).astype(bf),
            sdkc=sdkc,
        ))
    return in_maps


def kernel(**inputs):
    from concourse.bass_utils import run_bass_kernel_spmd

    nc = _get_nc()
    in_maps = _prep_in_maps(**inputs)
    res = run_bass_kernel_spmd(nc, in_maps, CORES).results
    out = np.empty((B, N, DIM), np.float32)
    for b in range(B):
        out[b] = (np.asarray(res[2 * b]["outT"]).astype(np.float32)
                  + np.asarray(res[2 * b + 1]["outT"]).astype(np.float32)).T
    return out
